# revision 16
# baseline (speedup 1.0000x reference)
"""Trainium2 Bass kernel for nn_ODE4: explicit-Euler neural ODE + MLP head.

  y_{t+1} = y_t + dt_t * (tanh([y_t, e_t] @ Wr1 + br1) @ Wr2 + br2)
  out     = relu(preds @ W1 + b1) @ W2 + b2          # preds = [y_0..y_{T-1}]

Sharding: pure data parallel over batch B across 8 cores (128 rows each);
tiny weights replicated; the sequential scan over T stays local per core.

On-chip layout is feature-major ([S|H, batch] on partitions) so the tiny
contractions run on the PE. All y_t / e_t slices live at partition base 0
(a PE requirement), free-dim packed: chunk tiles [8, TC*128], slot t at
free offset 128*t.

  per step:  psum_h  = Wy^T y_t + We^T e_t   (2 matmuls, K=8)
             h       = tanh(psum_h + br1)    (ACT, per-partition bias)
             psum_f  = Wr2^T h (+ br2)       (matmul, K=32)
             y_{t+1} = (psum_f * dt_t) + y_t (fused DVE scalar_tensor_tensor)

x arrives [B, T, E] batch-major; PE transposes ([128,8] -> [8,128] into a
free-packed PSUM bank) produce the e-slots, DVE copies them to SBUF.

Head (bulk, overlapped with the scan):
  pre1[10,B] = W1^T y_t            -> relu+bias b1 (DVE tensor_scalar)
  out[B,2]   = u_t^T @ W2  with u_t as the stationary operand, free-packed
               into a [128, 2*TC] PSUM tile => already [b,(t,c)] for the DMA.
"""

import numpy as np
from contextlib import ExitStack

import concourse.bass as bass
import concourse.bacc as bacc
import concourse.mybir as mybir
from concourse.tile import TileContext
from concourse import bass_utils

F32 = mybir.dt.float32
AF = mybir.ActivationFunctionType
ALU = mybir.AluOpType

B, T, S, E, H = 1024, 4096, 8, 8, 32
NCORES = 8
BC = B // NCORES  # 128 per-core batch rows = matmul free dim


def build_ode_nc(T=T, TC=64, with_br2=False):
    """Emit the per-core Bass program. All cores run the same code (SPMD)."""
    assert TC % 4 == 0 and T % TC == 0
    nchunks = T // TC

    nc = bacc.Bacc()
    xs_d = nc.dram_tensor("xs", [BC, T * E], F32, kind="ExternalInput")
    y0t_d = nc.dram_tensor("y0t", [S, BC], F32, kind="ExternalInput")
    dtb_d = nc.dram_tensor("dtb", [S, T], F32, kind="ExternalInput")
    wy_d = nc.dram_tensor("wy", [S, H], F32, kind="ExternalInput")
    we_d = nc.dram_tensor("we", [E, H], F32, kind="ExternalInput")
    wr2_d = nc.dram_tensor("wr2", [H, S], F32, kind="ExternalInput")
    br1_d = nc.dram_tensor("br1c", [H, 1], F32, kind="ExternalInput")
    w1_d = nc.dram_tensor("w1", [S, 10], F32, kind="ExternalInput")
    w2_d = nc.dram_tensor("w2", [10, 2], F32, kind="ExternalInput")
    ident_d = nc.dram_tensor("ident", [128, 128], F32, kind="ExternalInput")
    if with_br2:
        br2_d = nc.dram_tensor("br2r", [1, S], F32, kind="ExternalInput")
    b1_d = nc.dram_tensor("b1c", [10, 1], F32, kind="ExternalInput")
    out_d = nc.dram_tensor("out", [BC, T * 2], F32, kind="ExternalOutput")

    with TileContext(nc) as tc, ExitStack() as ctx:
        cpool = ctx.enter_context(tc.tile_pool(name="consts", bufs=1))
        xbp = ctx.enter_context(tc.tile_pool(name="xb", bufs=2))
        xep = ctx.enter_context(tc.tile_pool(name="xe", bufs=2))
        ysp = ctx.enter_context(tc.tile_pool(name="ys", bufs=2))
        hp = ctx.enter_context(tc.tile_pool(name="h", bufs=3))
        up = ctx.enter_context(tc.tile_pool(name="u", bufs=3))
        osbp = ctx.enter_context(tc.tile_pool(name="osb", bufs=2))
        psp = ctx.enter_context(tc.tile_pool(name="psp", bufs=2, space="PSUM"))
        pup = ctx.enter_context(tc.tile_pool(name="pup", bufs=2, space="PSUM"))
        ptp = ctx.enter_context(tc.tile_pool(name="ptp", bufs=2, space="PSUM"))
        pop = ctx.enter_context(tc.tile_pool(name="pop", bufs=2, space="PSUM"))

        def cload(name, shape, dram):
            t_ = cpool.tile(shape, F32, tag=name)
            nc.sync.dma_start(t_[:], dram[:])
            return t_

        wy_t = cload("wy", [S, H], wy_d)
        we_t = cload("we", [E, H], we_d)
        wr2_t = cload("wr2", [H, S], wr2_d)
        br1_t = cload("br1", [H, 1], br1_d)
        w1_t = cload("w1", [S, 10], w1_d)
        w2_t = cload("w2", [10, 2], w2_d)
        id_t = cload("ident", [128, 128], ident_d)
        dt_t = cload("dtb", [S, T], dtb_d)
        b1_t = cload("b1", [10, 1], b1_d)
        if with_br2:
            br2_t = cload("br2", [1, S], br2_d)
            ones_t = cpool.tile([1, 128], F32, tag="ones")
            nc.gpsimd.memset(ones_t[:], 1.0)

        ys_tiles = []

        def new_ys_tile():
            t_ = ysp.tile([S, TC * 128], F32, tag="ys")
            ys_tiles.append(t_)
            return t_

        def yslot(g):
            """AP of y_g: [8, 128] at free offset 128*(g%TC)."""
            c, s = divmod(g, TC)
            return ys_tiles[c][:, 128 * s:128 * (s + 1)]

        ys0 = new_ys_tile()
        nc.sync.dma_start(ys0[:, 0:128], y0t_d[:])

        for c in range(nchunks):
            # ---- PRE: load + transpose x chunk into free-packed e-slots ----
            xb_t = xbp.tile([128, TC * E], F32, tag="xb")
            nc.sync.dma_start(xb_t[:], xs_d[:, c * TC * E:(c + 1) * TC * E])
            xe_t = xep.tile([S, TC * 128], F32, tag="xe")
            for blk in range(TC // 4):
                ptile = ptp.tile([S, 512], F32, tag="pt", space="PSUM")
                for k in range(4):
                    s = 4 * blk + k
                    nc.tensor.transpose(ptile[:, 128 * k:128 * (k + 1)],
                                        xb_t[:, 8 * s:8 * s + 8], id_t[:])
                nc.vector.tensor_copy(xe_t[:, 512 * blk:512 * (blk + 1)],
                                      ptile[:])

            def eslot(s):
                return xe_t[:, 128 * s:128 * (s + 1)]

            # ---- SCAN over this chunk ----
            for s in range(TC):
                g = c * TC + s
                if g >= T - 1:
                    break
                if g + 1 >= len(ys_tiles) * TC:
                    new_ys_tile()
                ya = yslot(g)
                ph = psp.tile([H, 128], F32, tag="sp", space="PSUM")
                nc.tensor.matmul(ph[:], wy_t[:], ya, start=True, stop=False)
                nc.tensor.matmul(ph[:], we_t[:], eslot(s),
                                 start=False, stop=True)
                h_t = hp.tile([H, 128], F32, tag="h")
                nc.scalar.activation(h_t[:], ph[:], AF.Tanh, bias=br1_t[:])
                pf = psp.tile([S, 128], F32, tag="sp", space="PSUM")
                nc.tensor.matmul(pf[:], wr2_t[:], h_t[:], start=True,
                                 stop=not with_br2)
                if with_br2:
                    nc.tensor.matmul(pf[:], br2_t[:], ones_t[:],
                                     start=False, stop=True)
                nc.vector.scalar_tensor_tensor(
                    yslot(g + 1), pf[:], dt_t[:, g:g + 1], ya,
                    ALU.mult, ALU.add)

            # ---- POST: MLP head for all t in this chunk ----
            po = pop.tile([128, 2 * TC], F32, tag="po", space="PSUM")
            for q4 in range(TC // 4):
                pu_t = pup.tile([10, 512], F32, tag="pu", space="PSUM")
                for k in range(4):
                    s = 4 * q4 + k
                    nc.tensor.matmul(pu_t[:, 128 * k:128 * (k + 1)], w1_t[:],
                                     yslot(c * TC + s), start=True, stop=True)
                u_t = up.tile([10, 512], F32, tag="u")
                nc.vector.tensor_scalar(u_t[:], pu_t[:], b1_t[:], 0.0,
                                        ALU.add, ALU.max)
                for k in range(4):
                    s = 4 * q4 + k
                    nc.tensor.matmul(po[:, 2 * s:2 * s + 2],
                                     u_t[:, 128 * k:128 * (k + 1)], w2_t[:],
                                     start=True, stop=True)
            osb_t = osbp.tile([128, 2 * TC], F32, tag="osb")
            nc.vector.tensor_copy(osb_t[:], po[:])
            nc.sync.dma_start(out_d[:, 2 * c * TC:2 * (c + 1) * TC],
                              osb_t[:])

    nc.compile()
    return nc


def _prep_inputs(x, t, y0, Wr1, br1, Wr2, br2, W1, b1, W2, b2, T_=T):
    """Host-side: build per-core input maps."""
    x = np.ascontiguousarray(np.asarray(x, np.float32))
    dt = np.zeros((T_,), np.float32)
    dt[:T_ - 1] = np.diff(np.asarray(t, np.float32))
    dtb = np.broadcast_to(dt[None, :], (S, T_)).copy()
    Wr1 = np.asarray(Wr1, np.float32)
    common = {
        "dtb": dtb,
        "wy": np.ascontiguousarray(Wr1[:S]),
        "we": np.ascontiguousarray(Wr1[S:]),
        "wr2": np.ascontiguousarray(np.asarray(Wr2, np.float32)),
        "br1c": np.asarray(br1, np.float32).reshape(H, 1).copy(),
        "w1": np.ascontiguousarray(np.asarray(W1, np.float32)),
        "w2": np.ascontiguousarray(np.asarray(W2, np.float32)),
        "ident": np.eye(128, dtype=np.float32),
        "b1c": np.asarray(b1, np.float32).reshape(10, 1).copy(),
    }
    with_br2 = bool(np.any(np.asarray(br2) != 0))
    if with_br2:
        common["br2r"] = np.asarray(br2, np.float32).reshape(1, S).copy()
    y0 = np.asarray(y0, np.float32)
    in_maps = []
    for k in range(NCORES):
        sl = slice(k * BC, (k + 1) * BC)
        in_maps.append({
            "xs": x[sl].reshape(BC, T_ * E).copy(),
            "y0t": np.ascontiguousarray(y0[sl].T),
            **common,
        })
    return in_maps, with_br2


# ---------------------------------------------------------------------------
# v2: scan in pre-activation space. State p_t = Wy^T y_t + We^T e_t + br1
# lives in a persistent PSUM accumulator; each step is only
#   h = tanh(p)  (ACT) ;  p += dtW~^T h + We^T e_{t+1} - We^T e_t  (PE)
# so the serial chain is 2 hops (ACT -> PE -> ACT). p_t is copied out by DVE
# (off-chain) and the head consumes p via host-folded matrices:
#   pre1 = M1 p - (M1 We^T) e + (b1 - M1 br1),  M1 = W1^T pinv(Wy^T).
# ---------------------------------------------------------------------------


def build_ode_nc_v2(T=T, TC=32, with_br2=False):
    assert TC % 4 == 0 and T % TC == 0
    nchunks = T // TC

    nc = bacc.Bacc()
    xs_d = nc.dram_tensor("xs", [BC, T * E], F32, kind="ExternalInput")
    y0t_d = nc.dram_tensor("y0t", [S, BC], F32, kind="ExternalInput")
    dtw_d = nc.dram_tensor("dtw", [H, T * H], F32, kind="ExternalInput")
    wy_d = nc.dram_tensor("wy", [S, H], F32, kind="ExternalInput")
    we_d = nc.dram_tensor("we", [E, H], F32, kind="ExternalInput")
    wem_d = nc.dram_tensor("wem", [E, H], F32, kind="ExternalInput")
    br1r_d = nc.dram_tensor("br1r", [1, H], F32, kind="ExternalInput")
    atl_d = nc.dram_tensor("atl", [H, 10], F32, kind="ExternalInput")
    bml_d = nc.dram_tensor("bml", [E, 10], F32, kind="ExternalInput")
    btc_d = nc.dram_tensor("btc", [10, 1], F32, kind="ExternalInput")
    w2_d = nc.dram_tensor("w2", [10, 2], F32, kind="ExternalInput")
    ident_d = nc.dram_tensor("ident", [128, 128], F32, kind="ExternalInput")
    if with_br2:
        dtbr2_d = nc.dram_tensor("dtbr2", [1, T * H], F32,
                                 kind="ExternalInput")
    out_d = nc.dram_tensor("out", [BC, T * 2], F32, kind="ExternalOutput")

    with TileContext(nc) as tc, ExitStack() as ctx:
        cpool = ctx.enter_context(tc.tile_pool(name="consts", bufs=1))
        dbr2p = ctx.enter_context(tc.tile_pool(name="dbr2p", bufs=3))
        xbp = ctx.enter_context(tc.tile_pool(name="xb", bufs=3))
        xep = ctx.enter_context(tc.tile_pool(name="xe", bufs=3))
        psb = ctx.enter_context(tc.tile_pool(name="psb", bufs=2))
        dtwp = ctx.enter_context(tc.tile_pool(name="dtwp", bufs=3))
        hp = ctx.enter_context(tc.tile_pool(name="h", bufs=3))
        up = ctx.enter_context(tc.tile_pool(name="u", bufs=3))
        osbp = ctx.enter_context(tc.tile_pool(name="osb", bufs=2))
        ppp = ctx.enter_context(tc.tile_pool(name="ppp", bufs=1, space="PSUM"))
        pup = ctx.enter_context(tc.tile_pool(name="pup", bufs=2, space="PSUM"))
        ptp = ctx.enter_context(tc.tile_pool(name="ptp", bufs=2, space="PSUM"))
        pop = ctx.enter_context(tc.tile_pool(name="pop", bufs=2, space="PSUM"))

        def cload(name, shape, dram):
            t_ = cpool.tile(shape, F32, tag=name)
            nc.sync.dma_start(t_[:], dram[:])
            return t_

        wy_t = cload("wy", [S, H], wy_d)
        we_t = cload("we", [E, H], we_d)
        wem_t = cload("wem", [E, H], wem_d)
        br1r_t = cload("br1r", [1, H], br1r_d)
        atl_t = cload("atl", [H, 10], atl_d)
        bml_t = cload("bml", [E, 10], bml_d)
        btc_t = cload("btc", [10, 1], btc_d)
        w2_t = cload("w2", [10, 2], w2_d)
        id_t = cload("ident", [128, 128], ident_d)
        y0s_t = cload("y0s", [S, BC], y0t_d)
        ones_t = cpool.tile([1, 128], F32, tag="ones")
        nc.gpsimd.memset(ones_t[:], 1.0)

        pp_t = ppp.tile([H, 128], F32, tag="pp", name="pp", space="PSUM")

        xe_tiles, ps_tiles, dtw_tiles, dtbr2_tiles = [], [], [], []

        def pre(c):
            xb_t = xbp.tile([128, TC * E], F32, tag="xb")
            nc.sync.dma_start(xb_t[:], xs_d[:, c * TC * E:(c + 1) * TC * E])
            xe_t = xep.tile([S, TC * 128], F32, tag="xe")
            for blk in range(TC // 4):
                ptile = ptp.tile([S, 512], F32, tag="pt", space="PSUM")
                for k in range(4):
                    s = 4 * blk + k
                    nc.tensor.transpose(ptile[:, 128 * k:128 * (k + 1)],
                                        xb_t[:, 8 * s:8 * s + 8], id_t[:])
                nc.vector.tensor_copy(xe_t[:, 512 * blk:512 * (blk + 1)],
                                      ptile[:])
            xe_tiles.append(xe_t)
            dtw_t = dtwp.tile([H, TC * H], F32, tag="dtw")
            nc.sync.dma_start(dtw_t[:],
                              dtw_d[:, c * TC * H:(c + 1) * TC * H])
            dtw_tiles.append(dtw_t)
            if with_br2:
                db_t = dbr2p.tile([1, TC * H], F32, tag="dbr2")
                nc.sync.dma_start(db_t[:],
                                  dtbr2_d[:, c * TC * H:(c + 1) * TC * H])
                dtbr2_tiles.append(db_t)

        def eslot(g):
            c, s = divmod(g, TC)
            return xe_tiles[c][:, 128 * s:128 * (s + 1)]

        pre(0)
        # p_0 = Wy^T y0 + We^T e_0 + br1
        nc.tensor.matmul(pp_t[:], wy_t[:], y0s_t[:], start=True, stop=False,
                         skip_group_check=True)
        nc.tensor.matmul(pp_t[:], we_t[:], eslot(0), start=False, stop=False,
                         skip_group_check=True)
        nc.tensor.matmul(pp_t[:], br1r_t[:], ones_t[:],
                         start=False, stop=True, skip_group_check=True)

        for c in range(nchunks):
            if c + 1 < nchunks:
                pre(c + 1)
            ps_t = psb.tile([H, TC * 128], F32, tag="ps")
            ps_tiles.append(ps_t)

            # ---- SCAN ----
            for s in range(TC):
                g = c * TC + s
                nc.vector.tensor_copy(ps_t[:, 128 * s:128 * (s + 1)],
                                      pp_t[:])
                if g >= T - 1:
                    break
                h_t = hp.tile([H, 128], F32, tag="h")
                nc.scalar.activation(h_t[:], pp_t[:], AF.Tanh)
                nc.tensor.matmul(pp_t[:], we_t[:], eslot(g + 1),
                                 start=False, stop=False,
                                 skip_group_check=True)
                nc.tensor.matmul(pp_t[:], wem_t[:], eslot(g),
                                 start=False, stop=False,
                                 skip_group_check=True)
                if with_br2:
                    nc.tensor.matmul(pp_t[:],
                                     dtbr2_tiles[c][:, H * s:H * (s + 1)],
                                     ones_t[:], start=False, stop=False,
                                     skip_group_check=True)
                nc.tensor.matmul(pp_t[:],
                                 dtw_tiles[c][:, H * s:H * (s + 1)],
                                 h_t[:], start=False, stop=True,
                                 skip_group_check=True)

            # ---- POST: head from stored p and e ----
            po = pop.tile([128, 2 * TC], F32, tag="po", space="PSUM")
            for q4 in range(TC // 4):
                pu_t = pup.tile([10, 512], F32, tag="pu", space="PSUM")
                for k in range(4):
                    s = 4 * q4 + k
                    g = c * TC + s
                    nc.tensor.matmul(pu_t[:, 128 * k:128 * (k + 1)],
                                     atl_t[:], ps_t[:, 128 * s:128 * (s + 1)],
                                     start=True, stop=False)
                    nc.tensor.matmul(pu_t[:, 128 * k:128 * (k + 1)],
                                     bml_t[:], eslot(g),
                                     start=False, stop=True)
                u_t = up.tile([10, 512], F32, tag="u")
                nc.vector.tensor_scalar(u_t[:], pu_t[:], btc_t[:], 0.0,
                                        ALU.add, ALU.max)
                for k in range(4):
                    s = 4 * q4 + k
                    nc.tensor.matmul(po[:, 2 * s:2 * s + 2],
                                     u_t[:, 128 * k:128 * (k + 1)], w2_t[:],
                                     start=True, stop=True)
            osb_t = osbp.tile([128, 2 * TC], F32, tag="osb")
            nc.vector.tensor_copy(osb_t[:], po[:])
            nc.sync.dma_start(out_d[:, 2 * c * TC:2 * (c + 1) * TC],
                              osb_t[:])

    nc.compile()
    return nc


def _prep_inputs_v2(x, t, y0, Wr1, br1, Wr2, br2, W1, b1, W2, b2, T_=T):
    x = np.ascontiguousarray(np.asarray(x, np.float32))
    dt = np.zeros((T_,), np.float32)
    dt[:T_ - 1] = np.diff(np.asarray(t, np.float32))
    Wr1 = np.asarray(Wr1, np.float32)
    Wy, We = Wr1[:S], Wr1[S:]
    Wr2 = np.asarray(Wr2, np.float32)
    W1 = np.asarray(W1, np.float32)
    br1 = np.asarray(br1, np.float32)
    Wt = (Wr2 @ Wy).astype(np.float32)                     # [H, H]
    dtw = (Wt[:, None, :] * dt[None, :, None]).astype(np.float32)
    M1 = (W1.T @ np.linalg.pinv(Wy.T.astype(np.float64))).astype(np.float32)
    common = {
        "dtw": np.ascontiguousarray(dtw.reshape(H, T_ * H)),
        "wy": np.ascontiguousarray(Wy),
        "we": np.ascontiguousarray(We),
        "wem": np.ascontiguousarray(-We),
        "br1r": br1.reshape(1, H).copy(),
        "atl": np.ascontiguousarray(M1.T),                 # [H, 10]
        "bml": np.ascontiguousarray(-(We @ M1.T)),         # [E, 10]
        "btc": (np.asarray(b1, np.float32)
                - M1 @ br1).reshape(10, 1).copy(),
        "w2": np.ascontiguousarray(np.asarray(W2, np.float32)),
        "ident": np.eye(128, dtype=np.float32),
    }
    with_br2 = bool(np.any(np.asarray(br2) != 0))
    if with_br2:
        wyb = (Wy.T.astype(np.float32)
               @ np.asarray(br2, np.float32).reshape(S))   # [H]
        dtbr2 = (wyb[None, None, :] * dt[None, :, None]).astype(np.float32)
        common["dtbr2"] = np.ascontiguousarray(dtbr2.reshape(1, T_ * H))
    y0 = np.asarray(y0, np.float32)
    in_maps = []
    for k in range(NCORES):
        sl = slice(k * BC, (k + 1) * BC)
        in_maps.append({
            "xs": x[sl].reshape(BC, T_ * E).copy(),
            "y0t": np.ascontiguousarray(y0[sl].T),
            **common,
        })
    return in_maps, with_br2


# ---------------------------------------------------------------------------
# v3: fp16 single-matmul scan with 1-step-lagged fine correction.
#
# State p_t = Wy^T y_t + We^T e_t + br1 lives in a persistent PSUM
# accumulator (fp32). Per step the serial chain is just
#     h = fp16(tanh(p))   (ACT, psum -> sbuf fp16)
#     p += Lm_t^T @ [h; d_c; d_f; 1]    (ONE fp16 matmul, K=49, N=128)
# where d = e_{t+1}-e_t is host-split into an fp16 pair (d_c, d_f) and
# Lm_t = fp16([dt*Wr2@Wy; We; We; dt*br2@Wy]). The fp16 weight-rounding
# residual Lf_t = fp16(exact - Lm_t) is applied by a second matmul on the
# SAME rhs slot, emitted one step late so it sits off the critical path
# (validated: end-to-end rel err ~1e-3 vs 2e-2 budget).
#
# Off-chain per step: DVE snapshots p -> fp16 ps for the head; head is
# pre1 = [atl; bml; btc]^T [ps; e; 1] (one fp16 mm per 4 steps, N=512),
# relu on DVE, then u-stationary mms into a batch-major [128, 2*TC] psum
# tile for a clean output DMA. x never touches the device: the host ships
# e / d-pairs / per-step weights as packed fp16 streams.
# ---------------------------------------------------------------------------

K_RHS = 49   # [h 32; d_c 8; d_f 8; ones 1]
K_PS = 41    # [ps 32; e 8; ones 1]


def build_ode_nc_v3(T=T, TC=64):
    assert TC % 4 == 0 and T % TC == 0
    nchunks = T // TC
    F16 = mybir.dt.float16

    nc = bacc.Bacc()
    rhsd_d = nc.dram_tensor("rhsd", [17, T * 128], F16, kind="ExternalInput")
    e16_d = nc.dram_tensor("e16", [9, T * 128], F16, kind="ExternalInput")
    lm_d = nc.dram_tensor("lm", [K_RHS, T * H], F16, kind="ExternalInput")
    lf_d = nc.dram_tensor("lf", [K_RHS, T * H], F16, kind="ExternalInput")
    lh_d = nc.dram_tensor("lh", [K_PS, 10], F16, kind="ExternalInput")
    w2_d = nc.dram_tensor("w2f", [10, 2], F16, kind="ExternalInput")
    p0_d = nc.dram_tensor("p0t", [H, BC], F32, kind="ExternalInput")
    id_d = nc.dram_tensor("id32", [H, H], F32, kind="ExternalInput")
    out_d = nc.dram_tensor("out", [BC, T * 2], F32, kind="ExternalOutput")

    with TileContext(nc) as tc, ExitStack() as ctx:
        cpool = ctx.enter_context(tc.tile_pool(name="consts", bufs=1))
        rhsp = ctx.enter_context(tc.tile_pool(name="rhs", bufs=3))
        psp = ctx.enter_context(tc.tile_pool(name="ps", bufs=2))
        lmp = ctx.enter_context(tc.tile_pool(name="lm", bufs=3))
        lfp = ctx.enter_context(tc.tile_pool(name="lf", bufs=3))
        usp = ctx.enter_context(tc.tile_pool(name="u", bufs=2))
        osbp = ctx.enter_context(tc.tile_pool(name="osb", bufs=2))
        ppp = ctx.enter_context(tc.tile_pool(name="ppp", bufs=1, space="PSUM"))
        pup = ctx.enter_context(tc.tile_pool(name="pup", bufs=2, space="PSUM"))
        pop = ctx.enter_context(tc.tile_pool(name="pop", bufs=2, space="PSUM"))

        def cload(name, shape, dram, dt_=F16):
            t_ = cpool.tile(shape, dt_, tag=name)
            nc.sync.dma_start(t_[:], dram[:])
            return t_

        lh_t = cload("lh", [K_PS, 10], lh_d)
        w2_t = cload("w2", [10, 2], w2_d)
        p0_t = cload("p0", [H, BC], p0_d, F32)
        id_t = cload("id32", [H, H], id_d, F32)

        pp = ppp.tile([H, 128], F32, tag="pp", name="pp", space="PSUM")

        rhs_tiles, lm_tiles, lf_tiles = [], [], []

        def pre(c):
            r = rhsp.tile([K_RHS, TC * 128], F16, tag="rhs")
            nc.sync.dma_start(r[32:49, :],
                              rhsd_d[:, c * TC * 128:(c + 1) * TC * 128])
            rhs_tiles.append(r)
            m = lmp.tile([K_RHS, TC * H], F16, tag="lm")
            nc.sync.dma_start(m[:], lm_d[:, c * TC * H:(c + 1) * TC * H])
            lm_tiles.append(m)
            fi = lfp.tile([K_RHS, TC * H], F16, tag="lf")
            nc.sync.dma_start(fi[:], lf_d[:, c * TC * H:(c + 1) * TC * H])
            lf_tiles.append(fi)

        def rslot(g):
            c, s = divmod(g, TC)
            return rhs_tiles[c][:, 128 * s:128 * (s + 1)]

        def hslot(g):
            c, s = divmod(g, TC)
            return rhs_tiles[c][0:32, 128 * s:128 * (s + 1)]

        def lmsl(g):
            c, s = divmod(g, TC)
            return lm_tiles[c][:, H * s:H * (s + 1)]

        def lfsl(g):
            c, s = divmod(g, TC)
            return lf_tiles[c][:, H * s:H * (s + 1)]

        pre(0)
        # p_0 = Wy^T y0 + We^T e_0 + br1, staged on host, injected via
        # an identity matmul (PE is the only PSUM writer).
        nc.tensor.matmul(pp[:], id_t[:], p0_t[:], start=True, stop=True,
                         skip_group_check=True)

        for c in range(nchunks):
            if c + 1 < nchunks:
                pre(c + 1)
            ps_t = psp.tile([K_PS, TC * 128], F16, tag="ps")
            nc.sync.dma_start(ps_t[32:41, :],
                              e16_d[:, c * TC * 128:(c + 1) * TC * 128])
            u_t = usp.tile([10, TC * 128], F16, tag="u")
            po = pop.tile([128, 2 * TC], F32, tag="po", space="PSUM")

            for s in range(TC):
                g = c * TC + s
                # snapshot p_g for the head (off-chain, parallel with tanh)
                nc.vector.tensor_copy(ps_t[0:32, 128 * s:128 * (s + 1)],
                                      pp[:])
                if g < T - 1:
                    nc.scalar.activation(hslot(g), pp[:], AF.Tanh)

                # head: one [41,10]x[41,512] mm per finished 4-slot block
                if s % 4 == 3:
                    b = s // 4
                    pu = pup.tile([10, 512], F32, tag="pu", space="PSUM")
                    nc.tensor.matmul(pu[:], lh_t[:],
                                     ps_t[:, 512 * b:512 * (b + 1)],
                                     start=True, stop=True)
                    nc.vector.tensor_scalar_max(
                        u_t[:, 512 * b:512 * (b + 1)], pu[:], 0.0)
                if s % 4 == 0 and s > 0:
                    b = s // 4 - 1
                    for k in range(4):
                        sl = 4 * b + k
                        nc.tensor.matmul(
                            po[:, 2 * sl:2 * sl + 2],
                            u_t[:, 128 * sl:128 * (sl + 1)], w2_t[:],
                            start=True, stop=True)

                # HAM-warming filler: keeps TensorE activity high so the
                # clock gate stays at 2.4 GHz; runs in the tanh-wait gap.
                pd = pdp.tile([128, 256], F32, tag="pd", space="PSUM")
                nc.tensor.matmul(pd[:], dum_t[:, 0:128], dum_t[:],
                                 start=True, stop=True)

                if g < T - 1:
                    if g >= 1:
                        nc.tensor.matmul(pp[:], lfsl(g - 1), rslot(g - 1),
                                         start=False, stop=False,
                                         skip_group_check=True)
                    nc.tensor.matmul(pp[:], lmsl(g), rslot(g),
                                     start=False, stop=True,
                                     skip_group_check=True)

            b = TC // 4 - 1
            for k in range(4):
                sl = 4 * b + k
                nc.tensor.matmul(po[:, 2 * sl:2 * sl + 2],
                                 u_t[:, 128 * sl:128 * (sl + 1)], w2_t[:],
                                 start=True, stop=True)
            osb_t = osbp.tile([128, 2 * TC], F32, tag="osb")
            nc.vector.tensor_copy(osb_t[:], po[:])
            nc.sync.dma_start(out_d[:, 2 * c * TC:2 * (c + 1) * TC],
                              osb_t[:])

    nc.compile()
    return nc


def _prep_inputs_v3(x, t, y0, Wr1, br1, Wr2, br2, W1, b1, W2, b2, T_=T):
    f16, f32, f64 = np.float16, np.float32, np.float64
    x = np.asarray(x, f32)
    tt = np.asarray(t, f32)
    y0 = np.asarray(y0, f32)
    Wr1 = np.asarray(Wr1, f32)
    Wy, We = Wr1[:S], Wr1[S:]
    Wr2 = np.asarray(Wr2, f32)
    br1 = np.asarray(br1, f32)
    br2 = np.asarray(br2, f32)
    W1 = np.asarray(W1, f32)
    b1 = np.asarray(b1, f32)
    W2 = np.asarray(W2, f32)
    dt = np.diff(tt).astype(f32)                      # [T-1]

    # shared lhsT streams [49, T*H]
    ex = np.zeros((T_, K_RHS, H), f64)
    Wt64 = f64(Wr2) @ f64(Wy)                         # [H, H]
    ex[:T_ - 1, 0:32] = dt[:, None, None].astype(f64) * Wt64[None]
    ex[:T_ - 1, 32:40] = f64(We)[None]
    ex[:T_ - 1, 40:48] = f64(We)[None]
    ex[:T_ - 1, 48] = dt[:, None].astype(f64) * (f64(br2) @ f64(Wy))[None]
    lm = ex.astype(f16)
    lf = (ex - lm.astype(f64)).astype(f16)
    lm_s = np.ascontiguousarray(lm.transpose(1, 0, 2).reshape(K_RHS, T_ * H))
    lf_s = np.ascontiguousarray(lf.transpose(1, 0, 2).reshape(K_RHS, T_ * H))

    # head lhsT [41, 10]
    M1 = f64(W1.T) @ np.linalg.pinv(f64(Wy.T))
    lh = np.zeros((K_PS, 10), f16)
    lh[0:32] = M1.T.astype(f16)
    lh[32:40] = (-(f64(We) @ M1.T)).astype(f16)
    lh[40] = (f64(b1) - M1 @ f64(br1)).astype(f16)

    common = {
        "lm": lm_s, "lf": lf_s, "lh": lh,
        "w2f": W2.astype(f16),
        "id32": np.eye(H, dtype=f32),
    }
    in_maps = []
    for k in range(NCORES):
        sl = slice(k * BC, (k + 1) * BC)
        eT = np.ascontiguousarray(x[sl].transpose(2, 1, 0))   # [E, T, BC]
        e16 = np.ones((9, T_, BC), f16)
        e16[0:8] = eT.astype(f16)
        d = eT[:, 1:, :] - eT[:, :-1, :]                      # f32 exact
        d_c = d.astype(f16)
        d_f = (d - d_c.astype(f32)).astype(f16)
        rhsd = np.ones((17, T_, BC), f16)
        rhsd[0:8, :T_ - 1] = d_c
        rhsd[0:8, T_ - 1] = 0
        rhsd[8:16, :T_ - 1] = d_f
        rhsd[8:16, T_ - 1] = 0
        p0 = (f64(Wy.T) @ f64(y0[sl].T) + f64(We.T) @ f64(eT[:, 0, :])
              + f64(br1)[:, None]).astype(f32)
        in_maps.append({
            "rhsd": rhsd.reshape(17, T_ * BC),
            "e16": e16.reshape(9, T_ * BC),
            "p0t": p0,
            **common,
        })
    return in_maps


# ---------------------------------------------------------------------------
# v4: v3 with the serial chain cut down to TANH -> one fp16 matmul.
#
# Two changes vs v3, both keeping rel err ~1.1e-3 (sim-validated):
#  * The per-step DVE snapshot serializes against the PE writes of the same
#    PSUM bank (HW bank hazard), putting its ~330ns on the chain. v4 keeps a
#    REPLICA accumulator ppB updated by a duplicate matmul right after the
#    main one; the head's snapshot CAST reads ppB, forming an independent
#    (lag-tolerant) PE->DVE chain off the critical path. ppB skips the fine
#    corrections: its drift affects only the readout head (~5e-4).
#  * The per-step fine-correction matmul is batched per QUARTET: one
#    [49,32]x[49,512] mm over 4 rhs slots with the quartet-mean residual
#    lhsT (dt varies ~1e-4 within a quartet -- negligible). The final
#    partial quartet is skipped entirely (matches sim).
# ---------------------------------------------------------------------------


def build_ode_nc_v4(T=T, TC=64):
    assert TC % 4 == 0 and T % TC == 0
    nchunks = T // TC
    F16 = mybir.dt.float16

    nc = bacc.Bacc()
    rhsd_d = nc.dram_tensor("rhsd", [17, T * 128], F16, kind="ExternalInput")
    e16_d = nc.dram_tensor("e16", [9, T * 128], F16, kind="ExternalInput")
    lm_d = nc.dram_tensor("lm", [K_RHS, T * H], F16, kind="ExternalInput")
    lq_d = nc.dram_tensor("lq", [K_RHS, (T // 4) * H], F16,
                          kind="ExternalInput")
    lh_d = nc.dram_tensor("lh", [K_PS, 10], F16, kind="ExternalInput")
    w2_d = nc.dram_tensor("w2f", [10, 2], F16, kind="ExternalInput")
    p0_d = nc.dram_tensor("p0t", [H, BC], F32, kind="ExternalInput")
    id_d = nc.dram_tensor("id32", [H, H], F32, kind="ExternalInput")
    out_d = nc.dram_tensor("out", [BC, T * 2], F32, kind="ExternalOutput")

    with TileContext(nc) as tc, ExitStack() as ctx:
        cpool = ctx.enter_context(tc.tile_pool(name="consts", bufs=1))
        rhsp = ctx.enter_context(tc.tile_pool(name="rhs", bufs=3))
        psp = ctx.enter_context(tc.tile_pool(name="ps", bufs=3))
        lmp = ctx.enter_context(tc.tile_pool(name="lm", bufs=3))
        lqp = ctx.enter_context(tc.tile_pool(name="lq", bufs=3))
        usp = ctx.enter_context(tc.tile_pool(name="u", bufs=2))
        osbp = ctx.enter_context(tc.tile_pool(name="osb", bufs=2))
        ppp = ctx.enter_context(tc.tile_pool(name="ppp", bufs=1, space="PSUM"))
        pup = ctx.enter_context(tc.tile_pool(name="pup", bufs=2, space="PSUM"))
        pop = ctx.enter_context(tc.tile_pool(name="pop", bufs=2, space="PSUM"))

        def cload(name, shape, dram, dt_=F16):
            t_ = cpool.tile(shape, dt_, tag=name)
            nc.sync.dma_start(t_[:], dram[:])
            return t_

        lh_t = cload("lh", [K_PS, 10], lh_d)
        w2_t = cload("w2", [10, 2], w2_d)
        p0_t = cload("p0", [H, BC], p0_d, F32)
        id_t = cload("id32", [H, H], id_d, F32)

        pp = ppp.tile([H, 128], F32, tag="pp", name="pp", space="PSUM")
        ppB = ppp.tile([H, 128], F32, tag="ppB", name="ppB", space="PSUM")

        rhs_tiles, ps_tiles, lm_tiles, lq_tiles = [], [], [], []

        def pre(c):
            r = rhsp.tile([K_RHS, TC * 128], F16, tag="rhs")
            nc.sync.dma_start(r[32:49, :],
                              rhsd_d[:, c * TC * 128:(c + 1) * TC * 128])
            rhs_tiles.append(r)
            m = lmp.tile([K_RHS, TC * H], F16, tag="lm")
            nc.sync.dma_start(m[:], lm_d[:, c * TC * H:(c + 1) * TC * H])
            lm_tiles.append(m)
            q = lqp.tile([K_RHS, (TC // 4) * H], F16, tag="lq")
            nc.sync.dma_start(
                q[:], lq_d[:, c * (TC // 4) * H:(c + 1) * (TC // 4) * H])
            lq_tiles.append(q)
            p_ = psp.tile([K_PS, TC * 128], F16, tag="ps")
            nc.sync.dma_start(p_[32:41, :],
                              e16_d[:, c * TC * 128:(c + 1) * TC * 128])
            ps_tiles.append(p_)

        def rslot(g, n=1):
            c, s = divmod(g, TC)
            return rhs_tiles[c][:, 128 * s:128 * (s + n)]

        def hslot(g):
            c, s = divmod(g, TC)
            return rhs_tiles[c][0:32, 128 * s:128 * (s + 1)]

        def psslot(g):
            c, s = divmod(g, TC)
            return ps_tiles[c][0:32, 128 * s:128 * (s + 1)]

        def lmsl(g):
            c, s = divmod(g, TC)
            return lm_tiles[c][:, H * s:H * (s + 1)]

        def lqsl(q):
            c, s = divmod(q, TC // 4)
            return lq_tiles[c][:, H * s:H * (s + 1)]

        pre(0)
        nc.tensor.matmul(pp[:], id_t[:], p0_t[:], start=True, stop=True,
                         skip_group_check=True)
        nc.tensor.matmul(ppB[:], id_t[:], p0_t[:], start=True, stop=True,
                         skip_group_check=True)
        nc.vector.tensor_copy(psslot(0), ppB[:])

        for c in range(nchunks):
            if c + 1 < nchunks:
                pre(c + 1)
            u_t = usp.tile([10, TC * 128], F16, tag="u")
            po = pop.tile([128, 2 * TC], F32, tag="po", space="PSUM")
            ps_t = ps_tiles[c]

            for s in range(TC):
                g = c * TC + s
                if g < T - 1:
                    nc.scalar.activation(hslot(g), pp[:], AF.Tanh)

                if s % 4 == 3:
                    b = s // 4
                    pu = pup.tile([10, 512], F32, tag="pu", space="PSUM")
                    nc.tensor.matmul(pu[:], lh_t[:],
                                     ps_t[:, 512 * b:512 * (b + 1)],
                                     start=True, stop=True)
                    nc.vector.tensor_scalar_max(
                        u_t[:, 512 * b:512 * (b + 1)], pu[:], 0.0)
                if s % 4 == 0 and s > 0:
                    b = s // 4 - 1
                    for k in range(4):
                        sl = 4 * b + k
                        nc.tensor.matmul(
                            po[:, 2 * sl:2 * sl + 2],
                            u_t[:, 128 * sl:128 * (sl + 1)], w2_t[:],
                            start=True, stop=True)

                # HAM-warming filler: keeps TensorE activity high so the
                # clock gate stays at 2.4 GHz; runs in the tanh-wait gap.
                pd = pdp.tile([128, 256], F32, tag="pd", space="PSUM")
                nc.tensor.matmul(pd[:], dum_t[:, 0:128], dum_t[:],
                                 start=True, stop=True)

                if g < T - 1:
                    if g % 4 == 3:
                        # quartet fine correction (A-state only)
                        nc.tensor.matmul(pp[:], lqsl(g // 4),
                                         rslot(g - 3, 4),
                                         start=False, stop=False,
                                         skip_group_check=True)
                    nc.tensor.matmul(pp[:], lmsl(g), rslot(g),
                                     start=False, stop=True,
                                     skip_group_check=True)
                    nc.tensor.matmul(ppB[:], lmsl(g), rslot(g),
                                     start=False, stop=True,
                                     skip_group_check=True)
                    nc.vector.tensor_copy(psslot(g + 1), ppB[:])

            b = TC // 4 - 1
            for k in range(4):
                sl = 4 * b + k
                nc.tensor.matmul(po[:, 2 * sl:2 * sl + 2],
                                 u_t[:, 128 * sl:128 * (sl + 1)], w2_t[:],
                                 start=True, stop=True)
            osb_t = osbp.tile([128, 2 * TC], F32, tag="osb")
            nc.vector.tensor_copy(osb_t[:], po[:])
            nc.sync.dma_start(out_d[:, 2 * c * TC:2 * (c + 1) * TC],
                              osb_t[:])

    nc.compile()
    return nc


def _prep_inputs_v4(x, t, y0, Wr1, br1, Wr2, br2, W1, b1, W2, b2, T_=T):
    f16, f32, f64 = np.float16, np.float32, np.float64
    x = np.asarray(x, f32)
    tt = np.asarray(t, f32)
    y0 = np.asarray(y0, f32)
    Wr1 = np.asarray(Wr1, f32)
    Wy, We = Wr1[:S], Wr1[S:]
    Wr2 = np.asarray(Wr2, f32)
    br1 = np.asarray(br1, f32)
    br2 = np.asarray(br2, f32)
    W1 = np.asarray(W1, f32)
    b1 = np.asarray(b1, f32)
    W2 = np.asarray(W2, f32)
    dt = np.diff(tt).astype(f32)

    ex = np.zeros((T_, K_RHS, H), f64)
    Wt64 = f64(Wr2) @ f64(Wy)
    ex[:T_ - 1, 0:32] = dt[:, None, None].astype(f64) * Wt64[None]
    ex[:T_ - 1, 32:40] = f64(We)[None]
    ex[:T_ - 1, 40:48] = f64(We)[None]
    ex[:T_ - 1, 48] = dt[:, None].astype(f64) * (f64(br2) @ f64(Wy))[None]
    lm = ex.astype(f16)
    resid = ex - lm.astype(f64)
    nq = T_ // 4
    lq = resid.reshape(nq, 4, K_RHS, H).mean(axis=1).astype(f16)
    lm_s = np.ascontiguousarray(lm.transpose(1, 0, 2).reshape(K_RHS, T_ * H))
    lq_s = np.ascontiguousarray(lq.transpose(1, 0, 2).reshape(K_RHS, nq * H))

    M1 = f64(W1.T) @ np.linalg.pinv(f64(Wy.T))
    lh = np.zeros((K_PS, 10), f16)
    lh[0:32] = M1.T.astype(f16)
    lh[32:40] = (-(f64(We) @ M1.T)).astype(f16)
    lh[40] = (f64(b1) - M1 @ f64(br1)).astype(f16)

    common = {
        "lm": lm_s, "lq": lq_s, "lh": lh,
        "w2f": W2.astype(f16),
        "id32": np.eye(H, dtype=f32),
    }
    in_maps = []
    for k in range(NCORES):
        sl = slice(k * BC, (k + 1) * BC)
        eT = np.ascontiguousarray(x[sl].transpose(2, 1, 0))
        e16 = np.ones((9, T_, BC), f16)
        e16[0:8] = eT.astype(f16)
        d = eT[:, 1:, :] - eT[:, :-1, :]
        d_c = d.astype(f16)
        d_f = (d - d_c.astype(f32)).astype(f16)
        rhsd = np.ones((17, T_, BC), f16)
        rhsd[0:8, :T_ - 1] = d_c
        rhsd[0:8, T_ - 1] = 0
        rhsd[8:16, :T_ - 1] = d_f
        rhsd[8:16, T_ - 1] = 0
        p0 = (f64(Wy.T) @ f64(y0[sl].T) + f64(We.T) @ f64(eT[:, 0, :])
              + f64(br1)[:, None]).astype(f32)
        in_maps.append({
            "rhsd": rhsd.reshape(17, T_ * BC),
            "e16": e16.reshape(9, T_ * BC),
            "p0t": p0,
            **common,
        })
    return in_maps


# ---------------------------------------------------------------------------
# v5: dual-accumulator design; the head becomes a second tiny PSUM state.
#
#   pp  [32,128] fp32: pre-activation scan state (as v3/v4)
#   ppH [10,128] fp32: pre1_t = W1^T y_t + b1, accumulated via
#        hm_t = Lfull_t @ M1^T  (the exogenous d-terms cancel exactly:
#        M1*dp contributes +M1 We^T d, the head needs -M1 We^T d)
#
# Serial chain is TANH -> fine-mm(lag-1) -> main-mm. The head state ppH is
# written by its own matmul and relu-snapshotted by DVE (psum->sbuf fp16,
# fused relu via tensor_scalar_max) on an independent lag-tolerant chain.
# No p snapshots, no e/lh streams, no pre1 matmuls.
# ---------------------------------------------------------------------------


def build_ode_nc_v5(T=T, TC=64):
    assert TC % 4 == 0 and T % TC == 0
    nchunks = T // TC
    F16 = mybir.dt.float16

    nc = bacc.Bacc()
    rhsd_d = nc.dram_tensor("rhsd", [17, T * 128], F16, kind="ExternalInput")
    lm_d = nc.dram_tensor("lm", [K_RHS, T * H], F16, kind="ExternalInput")
    lf_d = nc.dram_tensor("lf", [K_RHS, T * H], F16, kind="ExternalInput")
    hm_d = nc.dram_tensor("hm", [K_RHS, T * 10], F16, kind="ExternalInput")
    w2_d = nc.dram_tensor("w2f", [10, 2], F16, kind="ExternalInput")
    p0_d = nc.dram_tensor("p0t", [H, BC], F32, kind="ExternalInput")
    h0_d = nc.dram_tensor("pre10", [10, BC], F32, kind="ExternalInput")
    id_d = nc.dram_tensor("id32", [H, H], F32, kind="ExternalInput")
    idh_d = nc.dram_tensor("id10", [10, 10], F32, kind="ExternalInput")
    out_d = nc.dram_tensor("out", [BC, T * 2], F32, kind="ExternalOutput")

    with TileContext(nc) as tc, ExitStack() as ctx:
        cpool = ctx.enter_context(tc.tile_pool(name="consts", bufs=1))
        rhsp = ctx.enter_context(tc.tile_pool(name="rhs", bufs=3))
        lmp = ctx.enter_context(tc.tile_pool(name="lm", bufs=3))
        lfp = ctx.enter_context(tc.tile_pool(name="lf", bufs=3))
        hmp = ctx.enter_context(tc.tile_pool(name="hm", bufs=3))
        usp = ctx.enter_context(tc.tile_pool(name="u", bufs=3))
        osbp = ctx.enter_context(tc.tile_pool(name="osb", bufs=2))
        ppp = ctx.enter_context(tc.tile_pool(name="ppp", bufs=1, space="PSUM"))
        pop = ctx.enter_context(tc.tile_pool(name="pop", bufs=2, space="PSUM"))

        def cload(name, shape, dram, dt_=F16):
            t_ = cpool.tile(shape, dt_, tag=name)
            nc.sync.dma_start(t_[:], dram[:])
            return t_

        w2_t = cload("w2", [10, 2], w2_d)
        p0_t = cload("p0", [H, BC], p0_d, F32)
        h0_t = cload("h0", [10, BC], h0_d, F32)
        id_t = cload("id32", [H, H], id_d, F32)
        idh_t = cload("id10", [10, 10], idh_d, F32)

        pp = ppp.tile([H, 128], F32, tag="pp", name="pp", space="PSUM")
        ppH = ppp.tile([10, 128], F32, tag="ppH", name="ppH", space="PSUM")
        pdp = ctx.enter_context(tc.tile_pool(name="pdp", bufs=2, space="PSUM"))
        dum_t = cpool.tile([1, 256], F16, tag="dum")
        nc.vector.memset(dum_t[:], 1.0)

        rhs_tiles, lm_tiles, lf_tiles, hm_tiles, u_tiles = [], [], [], [], []

        def pre(c):
            r = rhsp.tile([K_RHS, TC * 128], F16, tag="rhs")
            nc.sync.dma_start(r[32:49, :],
                              rhsd_d[:, c * TC * 128:(c + 1) * TC * 128])
            rhs_tiles.append(r)
            m = lmp.tile([K_RHS, TC * H], F16, tag="lm")
            nc.sync.dma_start(m[:], lm_d[:, c * TC * H:(c + 1) * TC * H])
            lm_tiles.append(m)
            fi = lfp.tile([K_RHS, TC * H], F16, tag="lf")
            nc.sync.dma_start(fi[:], lf_d[:, c * TC * H:(c + 1) * TC * H])
            lf_tiles.append(fi)
            hh = hmp.tile([K_RHS, TC * 10], F16, tag="hm")
            nc.sync.dma_start(hh[:], hm_d[:, c * TC * 10:(c + 1) * TC * 10])
            hm_tiles.append(hh)
            u = usp.tile([10, TC * 128], F16, tag="u")
            u_tiles.append(u)

        def rslot(g):
            c, s = divmod(g, TC)
            return rhs_tiles[c][:, 128 * s:128 * (s + 1)]

        def hslot(g):
            c, s = divmod(g, TC)
            return rhs_tiles[c][0:32, 128 * s:128 * (s + 1)]

        def uslot(g):
            c, s = divmod(g, TC)
            return u_tiles[c][:, 128 * s:128 * (s + 1)]

        def lmsl(g):
            c, s = divmod(g, TC)
            return lm_tiles[c][:, H * s:H * (s + 1)]

        def lfsl(g):
            c, s = divmod(g, TC)
            return lf_tiles[c][:, H * s:H * (s + 1)]

        def hmsl(g):
            c, s = divmod(g, TC)
            return hm_tiles[c][:, 10 * s:10 * (s + 1)]

        pre(0)
        nc.tensor.matmul(pp[:], id_t[:], p0_t[:], start=True, stop=True,
                         skip_group_check=True)
        nc.tensor.matmul(ppH[:], idh_t[:], h0_t[:], start=True, stop=True,
                         skip_group_check=True)
        nc.vector.tensor_scalar_max(uslot(0), ppH[:], 0.0)

        for c in range(nchunks):
            if c + 1 < nchunks:
                pre(c + 1)
            u_t = u_tiles[c]
            po = pop.tile([128, 2 * TC], F32, tag="po", space="PSUM")

            for s in range(TC):
                g = c * TC + s
                if g < T - 1:
                    nc.scalar.activation(hslot(g), pp[:], AF.Tanh)

                if s % 4 == 0 and s > 0:
                    b = s // 4 - 1
                    for k in range(4):
                        sl = 4 * b + k
                        nc.tensor.matmul(
                            po[:, 2 * sl:2 * sl + 2],
                            u_t[:, 128 * sl:128 * (sl + 1)], w2_t[:],
                            start=True, stop=True)

                # HAM-warming filler: keeps TensorE activity high so the
                # clock gate stays at 2.4 GHz; runs in the tanh-wait gap.
                pd = pdp.tile([128, 256], F32, tag="pd", space="PSUM")
                nc.tensor.matmul(pd[:], dum_t[:, 0:128], dum_t[:],
                                 start=True, stop=True)

                if g < T - 1:
                    if g >= 1:
                        nc.tensor.matmul(pp[:], lfsl(g - 1), rslot(g - 1),
                                         start=False, stop=False,
                                         skip_group_check=True)
                    nc.tensor.matmul(pp[:], lmsl(g), rslot(g),
                                     start=False, stop=True,
                                     skip_group_check=True)
                    nc.tensor.matmul(ppH[:], hmsl(g), rslot(g),
                                     start=False, stop=True,
                                     skip_group_check=True)
                    nc.vector.tensor_scalar_max(uslot(g + 1), ppH[:], 0.0)

            b = TC // 4 - 1
            for k in range(4):
                sl = 4 * b + k
                nc.tensor.matmul(po[:, 2 * sl:2 * sl + 2],
                                 u_t[:, 128 * sl:128 * (sl + 1)], w2_t[:],
                                 start=True, stop=True)
            osb_t = osbp.tile([128, 2 * TC], F32, tag="osb")
            nc.vector.tensor_copy(osb_t[:], po[:])
            nc.sync.dma_start(out_d[:, 2 * c * TC:2 * (c + 1) * TC],
                              osb_t[:])

    nc.compile()
    return nc


def _prep_inputs_v5(x, t, y0, Wr1, br1, Wr2, br2, W1, b1, W2, b2, T_=T):
    f16, f32, f64 = np.float16, np.float32, np.float64
    x = np.asarray(x, f32)
    tt = np.asarray(t, f32)
    y0 = np.asarray(y0, f32)
    Wr1 = np.asarray(Wr1, f32)
    Wy, We = Wr1[:S], Wr1[S:]
    Wr2 = np.asarray(Wr2, f32)
    br1 = np.asarray(br1, f32)
    br2 = np.asarray(br2, f32)
    W1 = np.asarray(W1, f32)
    b1 = np.asarray(b1, f32)
    W2 = np.asarray(W2, f32)
    dt = np.diff(tt).astype(f32)

    ex = np.zeros((T_, K_RHS, H), f64)
    Wt64 = f64(Wr2) @ f64(Wy)
    ex[:T_ - 1, 0:32] = dt[:, None, None].astype(f64) * Wt64[None]
    ex[:T_ - 1, 32:40] = f64(We)[None]
    ex[:T_ - 1, 40:48] = f64(We)[None]
    ex[:T_ - 1, 48] = dt[:, None].astype(f64) * (f64(br2) @ f64(Wy))[None]
    lm = ex.astype(f16)
    lf = (ex - lm.astype(f64)).astype(f16)

    M1 = f64(W1.T) @ np.linalg.pinv(f64(Wy.T))
    hm = np.zeros((T_, K_RHS, 10), f64)
    hm[:, 0:32] = ex[:, 0:32] @ M1.T
    hm[:, 48] = ex[:, 48] @ M1.T
    hm = hm.astype(f16)

    lm_s = np.ascontiguousarray(lm.transpose(1, 0, 2).reshape(K_RHS, T_ * H))
    lf_s = np.ascontiguousarray(lf.transpose(1, 0, 2).reshape(K_RHS, T_ * H))
    hm_s = np.ascontiguousarray(hm.transpose(1, 0, 2).reshape(K_RHS, T_ * 10))

    common = {
        "lm": lm_s, "lf": lf_s, "hm": hm_s,
        "w2f": W2.astype(f16),
        "id32": np.eye(H, dtype=f32),
        "id10": np.eye(10, dtype=f32),
    }
    in_maps = []
    for k in range(NCORES):
        sl = slice(k * BC, (k + 1) * BC)
        eT = np.ascontiguousarray(x[sl].transpose(2, 1, 0))
        d = eT[:, 1:, :] - eT[:, :-1, :]
        d_c = d.astype(f16)
        d_f = (d - d_c.astype(f32)).astype(f16)
        rhsd = np.ones((17, T_, BC), f16)
        rhsd[0:8, :T_ - 1] = d_c
        rhsd[0:8, T_ - 1] = 0
        rhsd[8:16, :T_ - 1] = d_f
        rhsd[8:16, T_ - 1] = 0
        p0 = (f64(Wy.T) @ f64(y0[sl].T) + f64(We.T) @ f64(eT[:, 0, :])
              + f64(br1)[:, None]).astype(f32)
        pre10 = (M1 @ f64(p0) - M1 @ f64(We.T) @ f64(eT[:, 0, :])
                 - (M1 @ f64(br1))[:, None] + f64(b1)[:, None]).astype(f32)
        in_maps.append({
            "rhsd": rhsd.reshape(17, T_ * BC),
            "p0t": p0,
            "pre10": pre10,
            **common,
        })
    return in_maps


# ---------------------------------------------------------------------------
# v6: v5 with the fine-correction matmul MERGED into the main matmul.
#
# The rhs stack widens to 98 rows: [h_t; h_{t-1}; d-parts; d-parts-dup].
# h_{t-1} is replicated into rows 32:64 of the NEXT slot by an off-chain
# DVE partition-offset copy (validated on HW); the d-parts are host-
# duplicated in the DMA stream. The combined lhsT carries main weights
# plus the fp16 rounding residuals (h-residual one step lagged, d/bias
# residuals current) -- one matmul per step updates the scan state with
# full fine correction. Serial chain: TANH -> single MM.
# ---------------------------------------------------------------------------

K2 = 98  # [h 32; h_lag 32; ones 1; d_c 8; d_f 8; d_c' 8; d_f' 8; ones' 1]
KH = 65  # head matmul contracts [h; h_lag(zero-weighted); ones]


def build_ode_nc_v6(T=T, TC=64):
    assert TC % 4 == 0 and T % TC == 0
    nchunks = T // TC
    F16 = mybir.dt.float16

    nc = bacc.Bacc()
    rhsd_d = nc.dram_tensor("rhsd", [34, T * 128], F16, kind="ExternalInput")
    lm_d = nc.dram_tensor("lm", [K2, T * H], F16, kind="ExternalInput")
    hm_d = nc.dram_tensor("hm", [KH, T * 10], F16, kind="ExternalInput")
    w2_d = nc.dram_tensor("w2f", [10, 2], F16, kind="ExternalInput")
    p0_d = nc.dram_tensor("p0t", [H, BC], F32, kind="ExternalInput")
    h0_d = nc.dram_tensor("pre10", [10, BC], F32, kind="ExternalInput")
    id_d = nc.dram_tensor("id32", [H, H], F32, kind="ExternalInput")
    idh_d = nc.dram_tensor("id10", [10, 10], F32, kind="ExternalInput")
    out_d = nc.dram_tensor("out", [BC, T * 2], F32, kind="ExternalOutput")

    with TileContext(nc) as tc, ExitStack() as ctx:
        cpool = ctx.enter_context(tc.tile_pool(name="consts", bufs=1))
        rhsp = ctx.enter_context(tc.tile_pool(name="rhs", bufs=3))
        lmp = ctx.enter_context(tc.tile_pool(name="lm", bufs=3))
        hmp = ctx.enter_context(tc.tile_pool(name="hm", bufs=3))
        usp = ctx.enter_context(tc.tile_pool(name="u", bufs=3))
        osbp = ctx.enter_context(tc.tile_pool(name="osb", bufs=2))
        ppp = ctx.enter_context(tc.tile_pool(name="ppp", bufs=1, space="PSUM"))
        pop = ctx.enter_context(tc.tile_pool(name="pop", bufs=2, space="PSUM"))

        def cload(name, shape, dram, dt_=F16):
            t_ = cpool.tile(shape, dt_, tag=name)
            nc.sync.dma_start(t_[:], dram[:])
            return t_

        w2_t = cload("w2", [10, 2], w2_d)
        p0_t = cload("p0", [H, BC], p0_d, F32)
        h0_t = cload("h0", [10, BC], h0_d, F32)
        id_t = cload("id32", [H, H], id_d, F32)
        idh_t = cload("id10", [10, 10], idh_d, F32)

        pp = ppp.tile([H, 128], F32, tag="pp", name="pp", space="PSUM")
        ppH = ppp.tile([10, 128], F32, tag="ppH", name="ppH", space="PSUM")
        pdp = ctx.enter_context(tc.tile_pool(name="pdp", bufs=2, space="PSUM"))
        dum_t = cpool.tile([1, 256], F16, tag="dum")
        nc.vector.memset(dum_t[:], 1.0)

        rhs_tiles, lm_tiles, hm_tiles, u_tiles = [], [], [], []

        def pre(c):
            r = rhsp.tile([K2, TC * 128], F16, tag="rhs")
            nc.sync.dma_start(r[64:98, :],
                              rhsd_d[:, c * TC * 128:(c + 1) * TC * 128])
            rhs_tiles.append(r)
            m = lmp.tile([K2, TC * H], F16, tag="lm")
            nc.sync.dma_start(m[:], lm_d[:, c * TC * H:(c + 1) * TC * H])
            lm_tiles.append(m)
            hh = hmp.tile([KH, TC * 10], F16, tag="hm")
            nc.sync.dma_start(hh[:], hm_d[:, c * TC * 10:(c + 1) * TC * 10])
            hm_tiles.append(hh)
            u = usp.tile([10, TC * 128], F16, tag="u")
            u_tiles.append(u)

        def rslot(g):
            c, s = divmod(g, TC)
            return rhs_tiles[c][:, 128 * s:128 * (s + 1)]

        def hslot(g):
            c, s = divmod(g, TC)
            return rhs_tiles[c][0:32, 128 * s:128 * (s + 1)]

        def lagslot(g):
            c, s = divmod(g, TC)
            return rhs_tiles[c][32:64, 128 * s:128 * (s + 1)]

        def hdslot(g):
            c, s = divmod(g, TC)
            return rhs_tiles[c][0:KH, 128 * s:128 * (s + 1)]

        def uslot(g):
            c, s = divmod(g, TC)
            return u_tiles[c][:, 128 * s:128 * (s + 1)]

        def lmsl(g):
            c, s = divmod(g, TC)
            return lm_tiles[c][:, H * s:H * (s + 1)]

        def hmsl(g):
            c, s = divmod(g, TC)
            return hm_tiles[c][:, 10 * s:10 * (s + 1)]

        pre(0)
        nc.vector.memset(rhs_tiles[0][32:64, 0:128], 0.0)
        nc.tensor.matmul(pp[:], id_t[:], p0_t[:], start=True, stop=True,
                         skip_group_check=True)
        nc.tensor.matmul(ppH[:], idh_t[:], h0_t[:], start=True, stop=True,
                         skip_group_check=True)
        nc.vector.tensor_scalar_max(uslot(0), ppH[:], 0.0)

        for c in range(nchunks):
            if c + 1 < nchunks:
                pre(c + 1)
            u_t = u_tiles[c]
            po = pop.tile([128, 2 * TC], F32, tag="po", space="PSUM")

            for s in range(TC):
                g = c * TC + s
                if g < T - 1:
                    nc.scalar.activation(hslot(g), pp[:], AF.Tanh)
                    if g + 1 <= T - 2:
                        # replicate h_t into next slot's lag rows (off-chain)
                        nc.vector.tensor_copy(lagslot(g + 1), hslot(g))
                    # scan matmul first in the PE FIFO after TANH
                    nc.tensor.matmul(pp[:], lmsl(g), rslot(g),
                                     start=False, stop=True,
                                     skip_group_check=True)
                # head state update lags one step so it never delays MMm
                if 1 <= g:
                    nc.tensor.matmul(ppH[:], hmsl(g - 1), hdslot(g - 1),
                                     start=False, stop=True,
                                     skip_group_check=True)
                    nc.vector.tensor_scalar_max(uslot(g), ppH[:], 0.0)

                if s % 4 == 0 and s > 0:
                    b = s // 4 - 1
                    for k in range(4):
                        sl = 4 * b + k
                        nc.tensor.matmul(
                            po[:, 2 * sl:2 * sl + 2],
                            u_t[:, 128 * sl:128 * (sl + 1)], w2_t[:],
                            start=True, stop=True)

                # HAM-warming filler: keeps TensorE activity high so the
                # clock gate stays at 2.4 GHz; runs in the tanh-wait gap.
                pd = pdp.tile([128, 256], F32, tag="pd", space="PSUM")
                nc.tensor.matmul(pd[:], dum_t[:, 0:128], dum_t[:],
                                 start=True, stop=True)

            b = TC // 4 - 1
            for k in range(4):
                sl = 4 * b + k
                nc.tensor.matmul(po[:, 2 * sl:2 * sl + 2],
                                 u_t[:, 128 * sl:128 * (sl + 1)], w2_t[:],
                                 start=True, stop=True)
            osb_t = osbp.tile([128, 2 * TC], F32, tag="osb")
            nc.vector.tensor_copy(osb_t[:], po[:])
            nc.sync.dma_start(out_d[:, 2 * c * TC:2 * (c + 1) * TC],
                              osb_t[:])

    nc.compile()
    return nc


def _prep_inputs_v6(x, t, y0, Wr1, br1, Wr2, br2, W1, b1, W2, b2, T_=T):
    f16, f32, f64 = np.float16, np.float32, np.float64
    x = np.asarray(x, f32)
    tt = np.asarray(t, f32)
    y0 = np.asarray(y0, f32)
    Wr1 = np.asarray(Wr1, f32)
    Wy, We = Wr1[:S], Wr1[S:]
    Wr2 = np.asarray(Wr2, f32)
    br1 = np.asarray(br1, f32)
    br2 = np.asarray(br2, f32)
    W1 = np.asarray(W1, f32)
    b1 = np.asarray(b1, f32)
    W2 = np.asarray(W2, f32)
    dt = np.diff(tt).astype(f32)

    # exact per-step weights in the 49-row basis, then split main/residual
    ex = np.zeros((T_, K_RHS, H), f64)
    Wt64 = f64(Wr2) @ f64(Wy)
    ex[:T_ - 1, 0:32] = dt[:, None, None].astype(f64) * Wt64[None]
    ex[:T_ - 1, 32:40] = f64(We)[None]
    ex[:T_ - 1, 40:48] = f64(We)[None]
    ex[:T_ - 1, 48] = dt[:, None].astype(f64) * (f64(br2) @ f64(Wy))[None]
    main = ex.astype(f16)
    resid = (ex - main.astype(f64)).astype(f16)

    # rows: 0:32 h | 32:64 h_lag | 64 ones | 65:73 d_c | 73:81 d_f
    #       | 81:89 d_c' | 89:97 d_f' | 97 ones'
    lm = np.zeros((T_, K2, H), f16)
    lm[:, 0:32] = main[:, 0:32]
    lm[1:, 32:64] = resid[:T_ - 1, 0:32]     # h-residual, lagged one step
    lm[:, 64] = main[:, 48]
    lm[:, 65:73] = main[:, 32:40]
    lm[:, 73:81] = main[:, 40:48]
    lm[:, 81:89] = resid[:, 32:40]
    lm[:, 89:97] = resid[:, 40:48]
    lm[:, 97] = resid[:, 48]

    M1 = f64(W1.T) @ np.linalg.pinv(f64(Wy.T))
    hm = np.zeros((T_, KH, 10), f64)
    hm[:, 0:32] = ex[:, 0:32] @ M1.T
    hm[:, 64] = ex[:, 48] @ M1.T
    hm = hm.astype(f16)

    lm_s = np.ascontiguousarray(lm.transpose(1, 0, 2).reshape(K2, T_ * H))
    hm_s = np.ascontiguousarray(hm.transpose(1, 0, 2).reshape(KH, T_ * 10))

    common = {
        "lm": lm_s, "hm": hm_s,
        "w2f": W2.astype(f16),
        "id32": np.eye(H, dtype=f32),
        "id10": np.eye(10, dtype=f32),
    }
    in_maps = []
    for k in range(NCORES):
        sl = slice(k * BC, (k + 1) * BC)
        eT = np.ascontiguousarray(x[sl].transpose(2, 1, 0))
        d = eT[:, 1:, :] - eT[:, :-1, :]
        d_c = d.astype(f16)
        d_f = (d - d_c.astype(f32)).astype(f16)
        rhsd = np.ones((34, T_, BC), f16)
        for base in (1, 17):                  # dest rows 65:81 and 81:97
            rhsd[base:base + 8, :T_ - 1] = d_c
            rhsd[base:base + 8, T_ - 1] = 0
            rhsd[base + 8:base + 16, :T_ - 1] = d_f
            rhsd[base + 8:base + 16, T_ - 1] = 0
        p0 = (f64(Wy.T) @ f64(y0[sl].T) + f64(We.T) @ f64(eT[:, 0, :])
              + f64(br1)[:, None]).astype(f32)
        pre10 = (M1 @ f64(p0) - M1 @ f64(We.T) @ f64(eT[:, 0, :])
                 - (M1 @ f64(br1))[:, None] + f64(b1)[:, None]).astype(f32)
        in_maps.append({
            "rhsd": rhsd.reshape(34, T_ * BC),
            "p0t": p0,
            "pre10": pre10,
            **common,
        })
    return in_maps


_NC_CACHE = {}


def kernel(x, t, y0, Wr1, br1, Wr2, br2, W1, b1, W2, b2):
    in_maps = _prep_inputs_v6(
        x, t, y0, Wr1, br1, Wr2, br2, W1, b1, W2, b2)
    key = ("v6d",)
    if key not in _NC_CACHE:
        _NC_CACHE[key] = build_ode_nc_v6(T=T, TC=64)
    nc = _NC_CACHE[key]
    res = bass_utils.run_bass_kernel_spmd(nc, in_maps,
                                          core_ids=list(range(NCORES)))
    outs = [res.results[k]["out"].reshape(BC, T, 2) for k in range(NCORES)]
    out = np.concatenate(outs, axis=0)
    b2 = np.asarray(b2, np.float32)
    if np.any(b2 != 0):
        out = out + b2[None, None, :]
    return out.astype(np.float32)



# revision 18
# speedup vs baseline: 1.1693x; 1.1693x over previous
"""Trainium2 Bass kernel for nn_ODE4: explicit-Euler neural ODE + MLP head.

  y_{t+1} = y_t + dt_t * (tanh([y_t, e_t] @ Wr1 + br1) @ Wr2 + br2)
  out     = relu(preds @ W1 + b1) @ W2 + b2          # preds = [y_0..y_{T-1}]

Sharding: pure data parallel over batch B across 8 cores (128 rows each);
tiny weights replicated; the sequential scan over T stays local per core.

On-chip layout is feature-major ([S|H, batch] on partitions) so the tiny
contractions run on the PE. All y_t / e_t slices live at partition base 0
(a PE requirement), free-dim packed: chunk tiles [8, TC*128], slot t at
free offset 128*t.

  per step:  psum_h  = Wy^T y_t + We^T e_t   (2 matmuls, K=8)
             h       = tanh(psum_h + br1)    (ACT, per-partition bias)
             psum_f  = Wr2^T h (+ br2)       (matmul, K=32)
             y_{t+1} = (psum_f * dt_t) + y_t (fused DVE scalar_tensor_tensor)

x arrives [B, T, E] batch-major; PE transposes ([128,8] -> [8,128] into a
free-packed PSUM bank) produce the e-slots, DVE copies them to SBUF.

Head (bulk, overlapped with the scan):
  pre1[10,B] = W1^T y_t            -> relu+bias b1 (DVE tensor_scalar)
  out[B,2]   = u_t^T @ W2  with u_t as the stationary operand, free-packed
               into a [128, 2*TC] PSUM tile => already [b,(t,c)] for the DMA.
"""

import numpy as np
from contextlib import ExitStack

import concourse.bass as bass
import concourse.bacc as bacc
import concourse.mybir as mybir
from concourse.tile import TileContext
from concourse import bass_utils

F32 = mybir.dt.float32
AF = mybir.ActivationFunctionType
ALU = mybir.AluOpType

B, T, S, E, H = 1024, 4096, 8, 8, 32
NCORES = 8
BC = B // NCORES  # 128 per-core batch rows = matmul free dim


def build_ode_nc(T=T, TC=64, with_br2=False):
    """Emit the per-core Bass program. All cores run the same code (SPMD)."""
    assert TC % 4 == 0 and T % TC == 0
    nchunks = T // TC

    nc = bacc.Bacc()
    xs_d = nc.dram_tensor("xs", [BC, T * E], F32, kind="ExternalInput")
    y0t_d = nc.dram_tensor("y0t", [S, BC], F32, kind="ExternalInput")
    dtb_d = nc.dram_tensor("dtb", [S, T], F32, kind="ExternalInput")
    wy_d = nc.dram_tensor("wy", [S, H], F32, kind="ExternalInput")
    we_d = nc.dram_tensor("we", [E, H], F32, kind="ExternalInput")
    wr2_d = nc.dram_tensor("wr2", [H, S], F32, kind="ExternalInput")
    br1_d = nc.dram_tensor("br1c", [H, 1], F32, kind="ExternalInput")
    w1_d = nc.dram_tensor("w1", [S, 10], F32, kind="ExternalInput")
    w2_d = nc.dram_tensor("w2", [10, 2], F32, kind="ExternalInput")
    ident_d = nc.dram_tensor("ident", [128, 128], F32, kind="ExternalInput")
    if with_br2:
        br2_d = nc.dram_tensor("br2r", [1, S], F32, kind="ExternalInput")
    b1_d = nc.dram_tensor("b1c", [10, 1], F32, kind="ExternalInput")
    out_d = nc.dram_tensor("out", [BC, T * 2], F32, kind="ExternalOutput")

    with TileContext(nc) as tc, ExitStack() as ctx:
        cpool = ctx.enter_context(tc.tile_pool(name="consts", bufs=1))
        xbp = ctx.enter_context(tc.tile_pool(name="xb", bufs=2))
        xep = ctx.enter_context(tc.tile_pool(name="xe", bufs=2))
        ysp = ctx.enter_context(tc.tile_pool(name="ys", bufs=2))
        hp = ctx.enter_context(tc.tile_pool(name="h", bufs=3))
        up = ctx.enter_context(tc.tile_pool(name="u", bufs=3))
        osbp = ctx.enter_context(tc.tile_pool(name="osb", bufs=2))
        psp = ctx.enter_context(tc.tile_pool(name="psp", bufs=2, space="PSUM"))
        pup = ctx.enter_context(tc.tile_pool(name="pup", bufs=2, space="PSUM"))
        ptp = ctx.enter_context(tc.tile_pool(name="ptp", bufs=2, space="PSUM"))
        pop = ctx.enter_context(tc.tile_pool(name="pop", bufs=2, space="PSUM"))

        def cload(name, shape, dram):
            t_ = cpool.tile(shape, F32, tag=name)
            nc.sync.dma_start(t_[:], dram[:])
            return t_

        wy_t = cload("wy", [S, H], wy_d)
        we_t = cload("we", [E, H], we_d)
        wr2_t = cload("wr2", [H, S], wr2_d)
        br1_t = cload("br1", [H, 1], br1_d)
        w1_t = cload("w1", [S, 10], w1_d)
        w2_t = cload("w2", [10, 2], w2_d)
        id_t = cload("ident", [128, 128], ident_d)
        dt_t = cload("dtb", [S, T], dtb_d)
        b1_t = cload("b1", [10, 1], b1_d)
        if with_br2:
            br2_t = cload("br2", [1, S], br2_d)
            ones_t = cpool.tile([1, 128], F32, tag="ones")
            nc.gpsimd.memset(ones_t[:], 1.0)

        ys_tiles = []

        def new_ys_tile():
            t_ = ysp.tile([S, TC * 128], F32, tag="ys")
            ys_tiles.append(t_)
            return t_

        def yslot(g):
            """AP of y_g: [8, 128] at free offset 128*(g%TC)."""
            c, s = divmod(g, TC)
            return ys_tiles[c][:, 128 * s:128 * (s + 1)]

        ys0 = new_ys_tile()
        nc.sync.dma_start(ys0[:, 0:128], y0t_d[:])

        for c in range(nchunks):
            # ---- PRE: load + transpose x chunk into free-packed e-slots ----
            xb_t = xbp.tile([128, TC * E], F32, tag="xb")
            nc.sync.dma_start(xb_t[:], xs_d[:, c * TC * E:(c + 1) * TC * E])
            xe_t = xep.tile([S, TC * 128], F32, tag="xe")
            for blk in range(TC // 4):
                ptile = ptp.tile([S, 512], F32, tag="pt", space="PSUM")
                for k in range(4):
                    s = 4 * blk + k
                    nc.tensor.transpose(ptile[:, 128 * k:128 * (k + 1)],
                                        xb_t[:, 8 * s:8 * s + 8], id_t[:])
                nc.vector.tensor_copy(xe_t[:, 512 * blk:512 * (blk + 1)],
                                      ptile[:])

            def eslot(s):
                return xe_t[:, 128 * s:128 * (s + 1)]

            # ---- SCAN over this chunk ----
            for s in range(TC):
                g = c * TC + s
                if g >= T - 1:
                    break
                if g + 1 >= len(ys_tiles) * TC:
                    new_ys_tile()
                ya = yslot(g)
                ph = psp.tile([H, 128], F32, tag="sp", space="PSUM")
                nc.tensor.matmul(ph[:], wy_t[:], ya, start=True, stop=False)
                nc.tensor.matmul(ph[:], we_t[:], eslot(s),
                                 start=False, stop=True)
                h_t = hp.tile([H, 128], F32, tag="h")
                nc.scalar.activation(h_t[:], ph[:], AF.Tanh, bias=br1_t[:])
                pf = psp.tile([S, 128], F32, tag="sp", space="PSUM")
                nc.tensor.matmul(pf[:], wr2_t[:], h_t[:], start=True,
                                 stop=not with_br2)
                if with_br2:
                    nc.tensor.matmul(pf[:], br2_t[:], ones_t[:],
                                     start=False, stop=True)
                nc.vector.scalar_tensor_tensor(
                    yslot(g + 1), pf[:], dt_t[:, g:g + 1], ya,
                    ALU.mult, ALU.add)

            # ---- POST: MLP head for all t in this chunk ----
            po = pop.tile([128, 2 * TC], F32, tag="po", space="PSUM")
            for q4 in range(TC // 4):
                pu_t = pup.tile([10, 512], F32, tag="pu", space="PSUM")
                for k in range(4):
                    s = 4 * q4 + k
                    nc.tensor.matmul(pu_t[:, 128 * k:128 * (k + 1)], w1_t[:],
                                     yslot(c * TC + s), start=True, stop=True)
                u_t = up.tile([10, 512], F32, tag="u")
                nc.vector.tensor_scalar(u_t[:], pu_t[:], b1_t[:], 0.0,
                                        ALU.add, ALU.max)
                for k in range(4):
                    s = 4 * q4 + k
                    nc.tensor.matmul(po[:, 2 * s:2 * s + 2],
                                     u_t[:, 128 * k:128 * (k + 1)], w2_t[:],
                                     start=True, stop=True)
            osb_t = osbp.tile([128, 2 * TC], F32, tag="osb")
            nc.vector.tensor_copy(osb_t[:], po[:])
            nc.sync.dma_start(out_d[:, 2 * c * TC:2 * (c + 1) * TC],
                              osb_t[:])

    nc.compile()
    return nc


def _prep_inputs(x, t, y0, Wr1, br1, Wr2, br2, W1, b1, W2, b2, T_=T):
    """Host-side: build per-core input maps."""
    x = np.ascontiguousarray(np.asarray(x, np.float32))
    dt = np.zeros((T_,), np.float32)
    dt[:T_ - 1] = np.diff(np.asarray(t, np.float32))
    dtb = np.broadcast_to(dt[None, :], (S, T_)).copy()
    Wr1 = np.asarray(Wr1, np.float32)
    common = {
        "dtb": dtb,
        "wy": np.ascontiguousarray(Wr1[:S]),
        "we": np.ascontiguousarray(Wr1[S:]),
        "wr2": np.ascontiguousarray(np.asarray(Wr2, np.float32)),
        "br1c": np.asarray(br1, np.float32).reshape(H, 1).copy(),
        "w1": np.ascontiguousarray(np.asarray(W1, np.float32)),
        "w2": np.ascontiguousarray(np.asarray(W2, np.float32)),
        "ident": np.eye(128, dtype=np.float32),
        "b1c": np.asarray(b1, np.float32).reshape(10, 1).copy(),
    }
    with_br2 = bool(np.any(np.asarray(br2) != 0))
    if with_br2:
        common["br2r"] = np.asarray(br2, np.float32).reshape(1, S).copy()
    y0 = np.asarray(y0, np.float32)
    in_maps = []
    for k in range(NCORES):
        sl = slice(k * BC, (k + 1) * BC)
        in_maps.append({
            "xs": x[sl].reshape(BC, T_ * E).copy(),
            "y0t": np.ascontiguousarray(y0[sl].T),
            **common,
        })
    return in_maps, with_br2


# ---------------------------------------------------------------------------
# v2: scan in pre-activation space. State p_t = Wy^T y_t + We^T e_t + br1
# lives in a persistent PSUM accumulator; each step is only
#   h = tanh(p)  (ACT) ;  p += dtW~^T h + We^T e_{t+1} - We^T e_t  (PE)
# so the serial chain is 2 hops (ACT -> PE -> ACT). p_t is copied out by DVE
# (off-chain) and the head consumes p via host-folded matrices:
#   pre1 = M1 p - (M1 We^T) e + (b1 - M1 br1),  M1 = W1^T pinv(Wy^T).
# ---------------------------------------------------------------------------


def build_ode_nc_v2(T=T, TC=32, with_br2=False):
    assert TC % 4 == 0 and T % TC == 0
    nchunks = T // TC

    nc = bacc.Bacc()
    xs_d = nc.dram_tensor("xs", [BC, T * E], F32, kind="ExternalInput")
    y0t_d = nc.dram_tensor("y0t", [S, BC], F32, kind="ExternalInput")
    dtw_d = nc.dram_tensor("dtw", [H, T * H], F32, kind="ExternalInput")
    wy_d = nc.dram_tensor("wy", [S, H], F32, kind="ExternalInput")
    we_d = nc.dram_tensor("we", [E, H], F32, kind="ExternalInput")
    wem_d = nc.dram_tensor("wem", [E, H], F32, kind="ExternalInput")
    br1r_d = nc.dram_tensor("br1r", [1, H], F32, kind="ExternalInput")
    atl_d = nc.dram_tensor("atl", [H, 10], F32, kind="ExternalInput")
    bml_d = nc.dram_tensor("bml", [E, 10], F32, kind="ExternalInput")
    btc_d = nc.dram_tensor("btc", [10, 1], F32, kind="ExternalInput")
    w2_d = nc.dram_tensor("w2", [10, 2], F32, kind="ExternalInput")
    ident_d = nc.dram_tensor("ident", [128, 128], F32, kind="ExternalInput")
    if with_br2:
        dtbr2_d = nc.dram_tensor("dtbr2", [1, T * H], F32,
                                 kind="ExternalInput")
    out_d = nc.dram_tensor("out", [BC, T * 2], F32, kind="ExternalOutput")

    with TileContext(nc) as tc, ExitStack() as ctx:
        cpool = ctx.enter_context(tc.tile_pool(name="consts", bufs=1))
        dbr2p = ctx.enter_context(tc.tile_pool(name="dbr2p", bufs=3))
        xbp = ctx.enter_context(tc.tile_pool(name="xb", bufs=3))
        xep = ctx.enter_context(tc.tile_pool(name="xe", bufs=3))
        psb = ctx.enter_context(tc.tile_pool(name="psb", bufs=2))
        dtwp = ctx.enter_context(tc.tile_pool(name="dtwp", bufs=3))
        hp = ctx.enter_context(tc.tile_pool(name="h", bufs=3))
        up = ctx.enter_context(tc.tile_pool(name="u", bufs=3))
        osbp = ctx.enter_context(tc.tile_pool(name="osb", bufs=2))
        ppp = ctx.enter_context(tc.tile_pool(name="ppp", bufs=1, space="PSUM"))
        pup = ctx.enter_context(tc.tile_pool(name="pup", bufs=2, space="PSUM"))
        ptp = ctx.enter_context(tc.tile_pool(name="ptp", bufs=2, space="PSUM"))
        pop = ctx.enter_context(tc.tile_pool(name="pop", bufs=2, space="PSUM"))

        def cload(name, shape, dram):
            t_ = cpool.tile(shape, F32, tag=name)
            nc.sync.dma_start(t_[:], dram[:])
            return t_

        wy_t = cload("wy", [S, H], wy_d)
        we_t = cload("we", [E, H], we_d)
        wem_t = cload("wem", [E, H], wem_d)
        br1r_t = cload("br1r", [1, H], br1r_d)
        atl_t = cload("atl", [H, 10], atl_d)
        bml_t = cload("bml", [E, 10], bml_d)
        btc_t = cload("btc", [10, 1], btc_d)
        w2_t = cload("w2", [10, 2], w2_d)
        id_t = cload("ident", [128, 128], ident_d)
        y0s_t = cload("y0s", [S, BC], y0t_d)
        ones_t = cpool.tile([1, 128], F32, tag="ones")
        nc.gpsimd.memset(ones_t[:], 1.0)

        pp_t = ppp.tile([H, 128], F32, tag="pp", name="pp", space="PSUM")

        xe_tiles, ps_tiles, dtw_tiles, dtbr2_tiles = [], [], [], []

        def pre(c):
            xb_t = xbp.tile([128, TC * E], F32, tag="xb")
            nc.sync.dma_start(xb_t[:], xs_d[:, c * TC * E:(c + 1) * TC * E])
            xe_t = xep.tile([S, TC * 128], F32, tag="xe")
            for blk in range(TC // 4):
                ptile = ptp.tile([S, 512], F32, tag="pt", space="PSUM")
                for k in range(4):
                    s = 4 * blk + k
                    nc.tensor.transpose(ptile[:, 128 * k:128 * (k + 1)],
                                        xb_t[:, 8 * s:8 * s + 8], id_t[:])
                nc.vector.tensor_copy(xe_t[:, 512 * blk:512 * (blk + 1)],
                                      ptile[:])
            xe_tiles.append(xe_t)
            dtw_t = dtwp.tile([H, TC * H], F32, tag="dtw")
            nc.sync.dma_start(dtw_t[:],
                              dtw_d[:, c * TC * H:(c + 1) * TC * H])
            dtw_tiles.append(dtw_t)
            if with_br2:
                db_t = dbr2p.tile([1, TC * H], F32, tag="dbr2")
                nc.sync.dma_start(db_t[:],
                                  dtbr2_d[:, c * TC * H:(c + 1) * TC * H])
                dtbr2_tiles.append(db_t)

        def eslot(g):
            c, s = divmod(g, TC)
            return xe_tiles[c][:, 128 * s:128 * (s + 1)]

        pre(0)
        # p_0 = Wy^T y0 + We^T e_0 + br1
        nc.tensor.matmul(pp_t[:], wy_t[:], y0s_t[:], start=True, stop=False,
                         skip_group_check=True)
        nc.tensor.matmul(pp_t[:], we_t[:], eslot(0), start=False, stop=False,
                         skip_group_check=True)
        nc.tensor.matmul(pp_t[:], br1r_t[:], ones_t[:],
                         start=False, stop=True, skip_group_check=True)

        for c in range(nchunks):
            if c + 1 < nchunks:
                pre(c + 1)
            ps_t = psb.tile([H, TC * 128], F32, tag="ps")
            ps_tiles.append(ps_t)

            # ---- SCAN ----
            for s in range(TC):
                g = c * TC + s
                nc.vector.tensor_copy(ps_t[:, 128 * s:128 * (s + 1)],
                                      pp_t[:])
                if g >= T - 1:
                    break
                h_t = hp.tile([H, 128], F32, tag="h")
                nc.scalar.activation(h_t[:], pp_t[:], AF.Tanh)
                nc.tensor.matmul(pp_t[:], we_t[:], eslot(g + 1),
                                 start=False, stop=False,
                                 skip_group_check=True)
                nc.tensor.matmul(pp_t[:], wem_t[:], eslot(g),
                                 start=False, stop=False,
                                 skip_group_check=True)
                if with_br2:
                    nc.tensor.matmul(pp_t[:],
                                     dtbr2_tiles[c][:, H * s:H * (s + 1)],
                                     ones_t[:], start=False, stop=False,
                                     skip_group_check=True)
                nc.tensor.matmul(pp_t[:],
                                 dtw_tiles[c][:, H * s:H * (s + 1)],
                                 h_t[:], start=False, stop=True,
                                 skip_group_check=True)

            # ---- POST: head from stored p and e ----
            po = pop.tile([128, 2 * TC], F32, tag="po", space="PSUM")
            for q4 in range(TC // 4):
                pu_t = pup.tile([10, 512], F32, tag="pu", space="PSUM")
                for k in range(4):
                    s = 4 * q4 + k
                    g = c * TC + s
                    nc.tensor.matmul(pu_t[:, 128 * k:128 * (k + 1)],
                                     atl_t[:], ps_t[:, 128 * s:128 * (s + 1)],
                                     start=True, stop=False)
                    nc.tensor.matmul(pu_t[:, 128 * k:128 * (k + 1)],
                                     bml_t[:], eslot(g),
                                     start=False, stop=True)
                u_t = up.tile([10, 512], F32, tag="u")
                nc.vector.tensor_scalar(u_t[:], pu_t[:], btc_t[:], 0.0,
                                        ALU.add, ALU.max)
                for k in range(4):
                    s = 4 * q4 + k
                    nc.tensor.matmul(po[:, 2 * s:2 * s + 2],
                                     u_t[:, 128 * k:128 * (k + 1)], w2_t[:],
                                     start=True, stop=True)
            osb_t = osbp.tile([128, 2 * TC], F32, tag="osb")
            nc.vector.tensor_copy(osb_t[:], po[:])
            nc.sync.dma_start(out_d[:, 2 * c * TC:2 * (c + 1) * TC],
                              osb_t[:])

    nc.compile()
    return nc


def _prep_inputs_v2(x, t, y0, Wr1, br1, Wr2, br2, W1, b1, W2, b2, T_=T):
    x = np.ascontiguousarray(np.asarray(x, np.float32))
    dt = np.zeros((T_,), np.float32)
    dt[:T_ - 1] = np.diff(np.asarray(t, np.float32))
    Wr1 = np.asarray(Wr1, np.float32)
    Wy, We = Wr1[:S], Wr1[S:]
    Wr2 = np.asarray(Wr2, np.float32)
    W1 = np.asarray(W1, np.float32)
    br1 = np.asarray(br1, np.float32)
    Wt = (Wr2 @ Wy).astype(np.float32)                     # [H, H]
    dtw = (Wt[:, None, :] * dt[None, :, None]).astype(np.float32)
    M1 = (W1.T @ np.linalg.pinv(Wy.T.astype(np.float64))).astype(np.float32)
    common = {
        "dtw": np.ascontiguousarray(dtw.reshape(H, T_ * H)),
        "wy": np.ascontiguousarray(Wy),
        "we": np.ascontiguousarray(We),
        "wem": np.ascontiguousarray(-We),
        "br1r": br1.reshape(1, H).copy(),
        "atl": np.ascontiguousarray(M1.T),                 # [H, 10]
        "bml": np.ascontiguousarray(-(We @ M1.T)),         # [E, 10]
        "btc": (np.asarray(b1, np.float32)
                - M1 @ br1).reshape(10, 1).copy(),
        "w2": np.ascontiguousarray(np.asarray(W2, np.float32)),
        "ident": np.eye(128, dtype=np.float32),
    }
    with_br2 = bool(np.any(np.asarray(br2) != 0))
    if with_br2:
        wyb = (Wy.T.astype(np.float32)
               @ np.asarray(br2, np.float32).reshape(S))   # [H]
        dtbr2 = (wyb[None, None, :] * dt[None, :, None]).astype(np.float32)
        common["dtbr2"] = np.ascontiguousarray(dtbr2.reshape(1, T_ * H))
    y0 = np.asarray(y0, np.float32)
    in_maps = []
    for k in range(NCORES):
        sl = slice(k * BC, (k + 1) * BC)
        in_maps.append({
            "xs": x[sl].reshape(BC, T_ * E).copy(),
            "y0t": np.ascontiguousarray(y0[sl].T),
            **common,
        })
    return in_maps, with_br2


# ---------------------------------------------------------------------------
# v3: fp16 single-matmul scan with 1-step-lagged fine correction.
#
# State p_t = Wy^T y_t + We^T e_t + br1 lives in a persistent PSUM
# accumulator (fp32). Per step the serial chain is just
#     h = fp16(tanh(p))   (ACT, psum -> sbuf fp16)
#     p += Lm_t^T @ [h; d_c; d_f; 1]    (ONE fp16 matmul, K=49, N=128)
# where d = e_{t+1}-e_t is host-split into an fp16 pair (d_c, d_f) and
# Lm_t = fp16([dt*Wr2@Wy; We; We; dt*br2@Wy]). The fp16 weight-rounding
# residual Lf_t = fp16(exact - Lm_t) is applied by a second matmul on the
# SAME rhs slot, emitted one step late so it sits off the critical path
# (validated: end-to-end rel err ~1e-3 vs 2e-2 budget).
#
# Off-chain per step: DVE snapshots p -> fp16 ps for the head; head is
# pre1 = [atl; bml; btc]^T [ps; e; 1] (one fp16 mm per 4 steps, N=512),
# relu on DVE, then u-stationary mms into a batch-major [128, 2*TC] psum
# tile for a clean output DMA. x never touches the device: the host ships
# e / d-pairs / per-step weights as packed fp16 streams.
# ---------------------------------------------------------------------------

K_RHS = 49   # [h 32; d_c 8; d_f 8; ones 1]
K_PS = 41    # [ps 32; e 8; ones 1]


def build_ode_nc_v3(T=T, TC=64):
    assert TC % 4 == 0 and T % TC == 0
    nchunks = T // TC
    F16 = mybir.dt.float16

    nc = bacc.Bacc()
    rhsd_d = nc.dram_tensor("rhsd", [17, T * 128], F16, kind="ExternalInput")
    e16_d = nc.dram_tensor("e16", [9, T * 128], F16, kind="ExternalInput")
    lm_d = nc.dram_tensor("lm", [K_RHS, T * H], F16, kind="ExternalInput")
    lf_d = nc.dram_tensor("lf", [K_RHS, T * H], F16, kind="ExternalInput")
    lh_d = nc.dram_tensor("lh", [K_PS, 10], F16, kind="ExternalInput")
    w2_d = nc.dram_tensor("w2f", [10, 2], F16, kind="ExternalInput")
    p0_d = nc.dram_tensor("p0t", [H, BC], F32, kind="ExternalInput")
    id_d = nc.dram_tensor("id32", [H, H], F32, kind="ExternalInput")
    out_d = nc.dram_tensor("out", [BC, T * 2], F32, kind="ExternalOutput")

    with TileContext(nc) as tc, ExitStack() as ctx:
        cpool = ctx.enter_context(tc.tile_pool(name="consts", bufs=1))
        rhsp = ctx.enter_context(tc.tile_pool(name="rhs", bufs=3))
        psp = ctx.enter_context(tc.tile_pool(name="ps", bufs=2))
        lmp = ctx.enter_context(tc.tile_pool(name="lm", bufs=3))
        lfp = ctx.enter_context(tc.tile_pool(name="lf", bufs=3))
        usp = ctx.enter_context(tc.tile_pool(name="u", bufs=2))
        osbp = ctx.enter_context(tc.tile_pool(name="osb", bufs=2))
        ppp = ctx.enter_context(tc.tile_pool(name="ppp", bufs=1, space="PSUM"))
        pup = ctx.enter_context(tc.tile_pool(name="pup", bufs=2, space="PSUM"))
        pop = ctx.enter_context(tc.tile_pool(name="pop", bufs=2, space="PSUM"))

        def cload(name, shape, dram, dt_=F16):
            t_ = cpool.tile(shape, dt_, tag=name)
            nc.sync.dma_start(t_[:], dram[:])
            return t_

        lh_t = cload("lh", [K_PS, 10], lh_d)
        w2_t = cload("w2", [10, 2], w2_d)
        p0_t = cload("p0", [H, BC], p0_d, F32)
        id_t = cload("id32", [H, H], id_d, F32)

        pp = ppp.tile([H, 128], F32, tag="pp", name="pp", space="PSUM")

        rhs_tiles, lm_tiles, lf_tiles = [], [], []

        def pre(c):
            r = rhsp.tile([K_RHS, TC * 128], F16, tag="rhs")
            nc.sync.dma_start(r[32:49, :],
                              rhsd_d[:, c * TC * 128:(c + 1) * TC * 128])
            rhs_tiles.append(r)
            m = lmp.tile([K_RHS, TC * H], F16, tag="lm")
            nc.sync.dma_start(m[:], lm_d[:, c * TC * H:(c + 1) * TC * H])
            lm_tiles.append(m)
            fi = lfp.tile([K_RHS, TC * H], F16, tag="lf")
            nc.sync.dma_start(fi[:], lf_d[:, c * TC * H:(c + 1) * TC * H])
            lf_tiles.append(fi)

        def rslot(g):
            c, s = divmod(g, TC)
            return rhs_tiles[c][:, 128 * s:128 * (s + 1)]

        def hslot(g):
            c, s = divmod(g, TC)
            return rhs_tiles[c][0:32, 128 * s:128 * (s + 1)]

        def lmsl(g):
            c, s = divmod(g, TC)
            return lm_tiles[c][:, H * s:H * (s + 1)]

        def lfsl(g):
            c, s = divmod(g, TC)
            return lf_tiles[c][:, H * s:H * (s + 1)]

        pre(0)
        # p_0 = Wy^T y0 + We^T e_0 + br1, staged on host, injected via
        # an identity matmul (PE is the only PSUM writer).
        nc.tensor.matmul(pp[:], id_t[:], p0_t[:], start=True, stop=True,
                         skip_group_check=True)

        for c in range(nchunks):
            if c + 1 < nchunks:
                pre(c + 1)
            ps_t = psp.tile([K_PS, TC * 128], F16, tag="ps")
            nc.sync.dma_start(ps_t[32:41, :],
                              e16_d[:, c * TC * 128:(c + 1) * TC * 128])
            u_t = usp.tile([10, TC * 128], F16, tag="u")
            po = pop.tile([128, 2 * TC], F32, tag="po", space="PSUM")

            for s in range(TC):
                g = c * TC + s
                # snapshot p_g for the head (off-chain, parallel with tanh)
                nc.vector.tensor_copy(ps_t[0:32, 128 * s:128 * (s + 1)],
                                      pp[:])
                if g < T - 1:
                    nc.scalar.activation(hslot(g), pp[:], AF.Tanh)

                # head: one [41,10]x[41,512] mm per finished 4-slot block
                if s % 4 == 3:
                    b = s // 4
                    pu = pup.tile([10, 512], F32, tag="pu", space="PSUM")
                    nc.tensor.matmul(pu[:], lh_t[:],
                                     ps_t[:, 512 * b:512 * (b + 1)],
                                     start=True, stop=True)
                    nc.vector.tensor_scalar_max(
                        u_t[:, 512 * b:512 * (b + 1)], pu[:], 0.0)
                if s % 4 == 0 and s > 0:
                    b = s // 4 - 1
                    for k in range(4):
                        sl = 4 * b + k
                        nc.tensor.matmul(
                            po[:, 2 * sl:2 * sl + 2],
                            u_t[:, 128 * sl:128 * (sl + 1)], w2_t[:],
                            start=True, stop=True)


                if g < T - 1:
                    if g >= 1:
                        nc.tensor.matmul(pp[:], lfsl(g - 1), rslot(g - 1),
                                         start=False, stop=False,
                                         skip_group_check=True)
                    nc.tensor.matmul(pp[:], lmsl(g), rslot(g),
                                     start=False, stop=True,
                                     skip_group_check=True)

            b = TC // 4 - 1
            for k in range(4):
                sl = 4 * b + k
                nc.tensor.matmul(po[:, 2 * sl:2 * sl + 2],
                                 u_t[:, 128 * sl:128 * (sl + 1)], w2_t[:],
                                 start=True, stop=True)
            osb_t = osbp.tile([128, 2 * TC], F32, tag="osb")
            nc.vector.tensor_copy(osb_t[:], po[:])
            nc.sync.dma_start(out_d[:, 2 * c * TC:2 * (c + 1) * TC],
                              osb_t[:])

    nc.compile()
    return nc


def _prep_inputs_v3(x, t, y0, Wr1, br1, Wr2, br2, W1, b1, W2, b2, T_=T):
    f16, f32, f64 = np.float16, np.float32, np.float64
    x = np.asarray(x, f32)
    tt = np.asarray(t, f32)
    y0 = np.asarray(y0, f32)
    Wr1 = np.asarray(Wr1, f32)
    Wy, We = Wr1[:S], Wr1[S:]
    Wr2 = np.asarray(Wr2, f32)
    br1 = np.asarray(br1, f32)
    br2 = np.asarray(br2, f32)
    W1 = np.asarray(W1, f32)
    b1 = np.asarray(b1, f32)
    W2 = np.asarray(W2, f32)
    dt = np.diff(tt).astype(f32)                      # [T-1]

    # shared lhsT streams [49, T*H]
    ex = np.zeros((T_, K_RHS, H), f64)
    Wt64 = f64(Wr2) @ f64(Wy)                         # [H, H]
    ex[:T_ - 1, 0:32] = dt[:, None, None].astype(f64) * Wt64[None]
    ex[:T_ - 1, 32:40] = f64(We)[None]
    ex[:T_ - 1, 40:48] = f64(We)[None]
    ex[:T_ - 1, 48] = dt[:, None].astype(f64) * (f64(br2) @ f64(Wy))[None]
    lm = ex.astype(f16)
    lf = (ex - lm.astype(f64)).astype(f16)
    lm_s = np.ascontiguousarray(lm.transpose(1, 0, 2).reshape(K_RHS, T_ * H))
    lf_s = np.ascontiguousarray(lf.transpose(1, 0, 2).reshape(K_RHS, T_ * H))

    # head lhsT [41, 10]
    M1 = f64(W1.T) @ np.linalg.pinv(f64(Wy.T))
    lh = np.zeros((K_PS, 10), f16)
    lh[0:32] = M1.T.astype(f16)
    lh[32:40] = (-(f64(We) @ M1.T)).astype(f16)
    lh[40] = (f64(b1) - M1 @ f64(br1)).astype(f16)

    common = {
        "lm": lm_s, "lf": lf_s, "lh": lh,
        "w2f": W2.astype(f16),
        "id32": np.eye(H, dtype=f32),
    }
    in_maps = []
    for k in range(NCORES):
        sl = slice(k * BC, (k + 1) * BC)
        eT = np.ascontiguousarray(x[sl].transpose(2, 1, 0))   # [E, T, BC]
        e16 = np.ones((9, T_, BC), f16)
        e16[0:8] = eT.astype(f16)
        d = eT[:, 1:, :] - eT[:, :-1, :]                      # f32 exact
        d_c = d.astype(f16)
        d_f = (d - d_c.astype(f32)).astype(f16)
        rhsd = np.ones((17, T_, BC), f16)
        rhsd[0:8, :T_ - 1] = d_c
        rhsd[0:8, T_ - 1] = 0
        rhsd[8:16, :T_ - 1] = d_f
        rhsd[8:16, T_ - 1] = 0
        p0 = (f64(Wy.T) @ f64(y0[sl].T) + f64(We.T) @ f64(eT[:, 0, :])
              + f64(br1)[:, None]).astype(f32)
        in_maps.append({
            "rhsd": rhsd.reshape(17, T_ * BC),
            "e16": e16.reshape(9, T_ * BC),
            "p0t": p0,
            **common,
        })
    return in_maps


# ---------------------------------------------------------------------------
# v4: v3 with the serial chain cut down to TANH -> one fp16 matmul.
#
# Two changes vs v3, both keeping rel err ~1.1e-3 (sim-validated):
#  * The per-step DVE snapshot serializes against the PE writes of the same
#    PSUM bank (HW bank hazard), putting its ~330ns on the chain. v4 keeps a
#    REPLICA accumulator ppB updated by a duplicate matmul right after the
#    main one; the head's snapshot CAST reads ppB, forming an independent
#    (lag-tolerant) PE->DVE chain off the critical path. ppB skips the fine
#    corrections: its drift affects only the readout head (~5e-4).
#  * The per-step fine-correction matmul is batched per QUARTET: one
#    [49,32]x[49,512] mm over 4 rhs slots with the quartet-mean residual
#    lhsT (dt varies ~1e-4 within a quartet -- negligible). The final
#    partial quartet is skipped entirely (matches sim).
# ---------------------------------------------------------------------------


def build_ode_nc_v4(T=T, TC=64):
    assert TC % 4 == 0 and T % TC == 0
    nchunks = T // TC
    F16 = mybir.dt.float16

    nc = bacc.Bacc()
    rhsd_d = nc.dram_tensor("rhsd", [17, T * 128], F16, kind="ExternalInput")
    e16_d = nc.dram_tensor("e16", [9, T * 128], F16, kind="ExternalInput")
    lm_d = nc.dram_tensor("lm", [K_RHS, T * H], F16, kind="ExternalInput")
    lq_d = nc.dram_tensor("lq", [K_RHS, (T // 4) * H], F16,
                          kind="ExternalInput")
    lh_d = nc.dram_tensor("lh", [K_PS, 10], F16, kind="ExternalInput")
    w2_d = nc.dram_tensor("w2f", [10, 2], F16, kind="ExternalInput")
    p0_d = nc.dram_tensor("p0t", [H, BC], F32, kind="ExternalInput")
    id_d = nc.dram_tensor("id32", [H, H], F32, kind="ExternalInput")
    out_d = nc.dram_tensor("out", [BC, T * 2], F32, kind="ExternalOutput")

    with TileContext(nc) as tc, ExitStack() as ctx:
        cpool = ctx.enter_context(tc.tile_pool(name="consts", bufs=1))
        rhsp = ctx.enter_context(tc.tile_pool(name="rhs", bufs=3))
        psp = ctx.enter_context(tc.tile_pool(name="ps", bufs=3))
        lmp = ctx.enter_context(tc.tile_pool(name="lm", bufs=3))
        lqp = ctx.enter_context(tc.tile_pool(name="lq", bufs=3))
        usp = ctx.enter_context(tc.tile_pool(name="u", bufs=2))
        osbp = ctx.enter_context(tc.tile_pool(name="osb", bufs=2))
        ppp = ctx.enter_context(tc.tile_pool(name="ppp", bufs=1, space="PSUM"))
        pup = ctx.enter_context(tc.tile_pool(name="pup", bufs=2, space="PSUM"))
        pop = ctx.enter_context(tc.tile_pool(name="pop", bufs=2, space="PSUM"))

        def cload(name, shape, dram, dt_=F16):
            t_ = cpool.tile(shape, dt_, tag=name)
            nc.sync.dma_start(t_[:], dram[:])
            return t_

        lh_t = cload("lh", [K_PS, 10], lh_d)
        w2_t = cload("w2", [10, 2], w2_d)
        p0_t = cload("p0", [H, BC], p0_d, F32)
        id_t = cload("id32", [H, H], id_d, F32)

        pp = ppp.tile([H, 128], F32, tag="pp", name="pp", space="PSUM")
        ppB = ppp.tile([H, 128], F32, tag="ppB", name="ppB", space="PSUM")

        rhs_tiles, ps_tiles, lm_tiles, lq_tiles = [], [], [], []

        def pre(c):
            r = rhsp.tile([K_RHS, TC * 128], F16, tag="rhs")
            nc.sync.dma_start(r[32:49, :],
                              rhsd_d[:, c * TC * 128:(c + 1) * TC * 128])
            rhs_tiles.append(r)
            m = lmp.tile([K_RHS, TC * H], F16, tag="lm")
            nc.sync.dma_start(m[:], lm_d[:, c * TC * H:(c + 1) * TC * H])
            lm_tiles.append(m)
            q = lqp.tile([K_RHS, (TC // 4) * H], F16, tag="lq")
            nc.sync.dma_start(
                q[:], lq_d[:, c * (TC // 4) * H:(c + 1) * (TC // 4) * H])
            lq_tiles.append(q)
            p_ = psp.tile([K_PS, TC * 128], F16, tag="ps")
            nc.sync.dma_start(p_[32:41, :],
                              e16_d[:, c * TC * 128:(c + 1) * TC * 128])
            ps_tiles.append(p_)

        def rslot(g, n=1):
            c, s = divmod(g, TC)
            return rhs_tiles[c][:, 128 * s:128 * (s + n)]

        def hslot(g):
            c, s = divmod(g, TC)
            return rhs_tiles[c][0:32, 128 * s:128 * (s + 1)]

        def psslot(g):
            c, s = divmod(g, TC)
            return ps_tiles[c][0:32, 128 * s:128 * (s + 1)]

        def lmsl(g):
            c, s = divmod(g, TC)
            return lm_tiles[c][:, H * s:H * (s + 1)]

        def lqsl(q):
            c, s = divmod(q, TC // 4)
            return lq_tiles[c][:, H * s:H * (s + 1)]

        pre(0)
        nc.tensor.matmul(pp[:], id_t[:], p0_t[:], start=True, stop=True,
                         skip_group_check=True)
        nc.tensor.matmul(ppB[:], id_t[:], p0_t[:], start=True, stop=True,
                         skip_group_check=True)
        nc.vector.tensor_copy(psslot(0), ppB[:])

        for c in range(nchunks):
            if c + 1 < nchunks:
                pre(c + 1)
            u_t = usp.tile([10, TC * 128], F16, tag="u")
            po = pop.tile([128, 2 * TC], F32, tag="po", space="PSUM")
            ps_t = ps_tiles[c]

            for s in range(TC):
                g = c * TC + s
                if g < T - 1:
                    nc.scalar.activation(hslot(g), pp[:], AF.Tanh)

                if s % 4 == 3:
                    b = s // 4
                    pu = pup.tile([10, 512], F32, tag="pu", space="PSUM")
                    nc.tensor.matmul(pu[:], lh_t[:],
                                     ps_t[:, 512 * b:512 * (b + 1)],
                                     start=True, stop=True)
                    nc.vector.tensor_scalar_max(
                        u_t[:, 512 * b:512 * (b + 1)], pu[:], 0.0)
                if s % 4 == 0 and s > 0:
                    b = s // 4 - 1
                    for k in range(4):
                        sl = 4 * b + k
                        nc.tensor.matmul(
                            po[:, 2 * sl:2 * sl + 2],
                            u_t[:, 128 * sl:128 * (sl + 1)], w2_t[:],
                            start=True, stop=True)


                if g < T - 1:
                    if g % 4 == 3:
                        # quartet fine correction (A-state only)
                        nc.tensor.matmul(pp[:], lqsl(g // 4),
                                         rslot(g - 3, 4),
                                         start=False, stop=False,
                                         skip_group_check=True)
                    nc.tensor.matmul(pp[:], lmsl(g), rslot(g),
                                     start=False, stop=True,
                                     skip_group_check=True)
                    nc.tensor.matmul(ppB[:], lmsl(g), rslot(g),
                                     start=False, stop=True,
                                     skip_group_check=True)
                    nc.vector.tensor_copy(psslot(g + 1), ppB[:])

            b = TC // 4 - 1
            for k in range(4):
                sl = 4 * b + k
                nc.tensor.matmul(po[:, 2 * sl:2 * sl + 2],
                                 u_t[:, 128 * sl:128 * (sl + 1)], w2_t[:],
                                 start=True, stop=True)
            osb_t = osbp.tile([128, 2 * TC], F32, tag="osb")
            nc.vector.tensor_copy(osb_t[:], po[:])
            nc.sync.dma_start(out_d[:, 2 * c * TC:2 * (c + 1) * TC],
                              osb_t[:])

    nc.compile()
    return nc


def _prep_inputs_v4(x, t, y0, Wr1, br1, Wr2, br2, W1, b1, W2, b2, T_=T):
    f16, f32, f64 = np.float16, np.float32, np.float64
    x = np.asarray(x, f32)
    tt = np.asarray(t, f32)
    y0 = np.asarray(y0, f32)
    Wr1 = np.asarray(Wr1, f32)
    Wy, We = Wr1[:S], Wr1[S:]
    Wr2 = np.asarray(Wr2, f32)
    br1 = np.asarray(br1, f32)
    br2 = np.asarray(br2, f32)
    W1 = np.asarray(W1, f32)
    b1 = np.asarray(b1, f32)
    W2 = np.asarray(W2, f32)
    dt = np.diff(tt).astype(f32)

    ex = np.zeros((T_, K_RHS, H), f64)
    Wt64 = f64(Wr2) @ f64(Wy)
    ex[:T_ - 1, 0:32] = dt[:, None, None].astype(f64) * Wt64[None]
    ex[:T_ - 1, 32:40] = f64(We)[None]
    ex[:T_ - 1, 40:48] = f64(We)[None]
    ex[:T_ - 1, 48] = dt[:, None].astype(f64) * (f64(br2) @ f64(Wy))[None]
    lm = ex.astype(f16)
    resid = ex - lm.astype(f64)
    nq = T_ // 4
    lq = resid.reshape(nq, 4, K_RHS, H).mean(axis=1).astype(f16)
    lm_s = np.ascontiguousarray(lm.transpose(1, 0, 2).reshape(K_RHS, T_ * H))
    lq_s = np.ascontiguousarray(lq.transpose(1, 0, 2).reshape(K_RHS, nq * H))

    M1 = f64(W1.T) @ np.linalg.pinv(f64(Wy.T))
    lh = np.zeros((K_PS, 10), f16)
    lh[0:32] = M1.T.astype(f16)
    lh[32:40] = (-(f64(We) @ M1.T)).astype(f16)
    lh[40] = (f64(b1) - M1 @ f64(br1)).astype(f16)

    common = {
        "lm": lm_s, "lq": lq_s, "lh": lh,
        "w2f": W2.astype(f16),
        "id32": np.eye(H, dtype=f32),
    }
    in_maps = []
    for k in range(NCORES):
        sl = slice(k * BC, (k + 1) * BC)
        eT = np.ascontiguousarray(x[sl].transpose(2, 1, 0))
        e16 = np.ones((9, T_, BC), f16)
        e16[0:8] = eT.astype(f16)
        d = eT[:, 1:, :] - eT[:, :-1, :]
        d_c = d.astype(f16)
        d_f = (d - d_c.astype(f32)).astype(f16)
        rhsd = np.ones((17, T_, BC), f16)
        rhsd[0:8, :T_ - 1] = d_c
        rhsd[0:8, T_ - 1] = 0
        rhsd[8:16, :T_ - 1] = d_f
        rhsd[8:16, T_ - 1] = 0
        p0 = (f64(Wy.T) @ f64(y0[sl].T) + f64(We.T) @ f64(eT[:, 0, :])
              + f64(br1)[:, None]).astype(f32)
        in_maps.append({
            "rhsd": rhsd.reshape(17, T_ * BC),
            "e16": e16.reshape(9, T_ * BC),
            "p0t": p0,
            **common,
        })
    return in_maps


# ---------------------------------------------------------------------------
# v5: dual-accumulator design; the head becomes a second tiny PSUM state.
#
#   pp  [32,128] fp32: pre-activation scan state (as v3/v4)
#   ppH [10,128] fp32: pre1_t = W1^T y_t + b1, accumulated via
#        hm_t = Lfull_t @ M1^T  (the exogenous d-terms cancel exactly:
#        M1*dp contributes +M1 We^T d, the head needs -M1 We^T d)
#
# Serial chain is TANH -> fine-mm(lag-1) -> main-mm. The head state ppH is
# written by its own matmul and relu-snapshotted by DVE (psum->sbuf fp16,
# fused relu via tensor_scalar_max) on an independent lag-tolerant chain.
# No p snapshots, no e/lh streams, no pre1 matmuls.
# ---------------------------------------------------------------------------


def build_ode_nc_v5(T=T, TC=64):
    assert TC % 4 == 0 and T % TC == 0
    nchunks = T // TC
    F16 = mybir.dt.float16

    nc = bacc.Bacc()
    rhsd_d = nc.dram_tensor("rhsd", [17, T * 128], F16, kind="ExternalInput")
    lm_d = nc.dram_tensor("lm", [K_RHS, T * H], F16, kind="ExternalInput")
    lf_d = nc.dram_tensor("lf", [K_RHS, T * H], F16, kind="ExternalInput")
    hm_d = nc.dram_tensor("hm", [K_RHS, T * 10], F16, kind="ExternalInput")
    w2_d = nc.dram_tensor("w2f", [10, 2], F16, kind="ExternalInput")
    p0_d = nc.dram_tensor("p0t", [H, BC], F32, kind="ExternalInput")
    h0_d = nc.dram_tensor("pre10", [10, BC], F32, kind="ExternalInput")
    id_d = nc.dram_tensor("id32", [H, H], F32, kind="ExternalInput")
    idh_d = nc.dram_tensor("id10", [10, 10], F32, kind="ExternalInput")
    out_d = nc.dram_tensor("out", [BC, T * 2], F32, kind="ExternalOutput")

    with TileContext(nc) as tc, ExitStack() as ctx:
        cpool = ctx.enter_context(tc.tile_pool(name="consts", bufs=1))
        rhsp = ctx.enter_context(tc.tile_pool(name="rhs", bufs=3))
        lmp = ctx.enter_context(tc.tile_pool(name="lm", bufs=3))
        lfp = ctx.enter_context(tc.tile_pool(name="lf", bufs=3))
        hmp = ctx.enter_context(tc.tile_pool(name="hm", bufs=3))
        usp = ctx.enter_context(tc.tile_pool(name="u", bufs=3))
        osbp = ctx.enter_context(tc.tile_pool(name="osb", bufs=2))
        ppp = ctx.enter_context(tc.tile_pool(name="ppp", bufs=1, space="PSUM"))
        pop = ctx.enter_context(tc.tile_pool(name="pop", bufs=2, space="PSUM"))

        def cload(name, shape, dram, dt_=F16):
            t_ = cpool.tile(shape, dt_, tag=name)
            nc.sync.dma_start(t_[:], dram[:])
            return t_

        w2_t = cload("w2", [10, 2], w2_d)
        p0_t = cload("p0", [H, BC], p0_d, F32)
        h0_t = cload("h0", [10, BC], h0_d, F32)
        id_t = cload("id32", [H, H], id_d, F32)
        idh_t = cload("id10", [10, 10], idh_d, F32)

        pp = ppp.tile([H, 128], F32, tag="pp", name="pp", space="PSUM")
        ppH = ppp.tile([10, 128], F32, tag="ppH", name="ppH", space="PSUM")
        pdp = ctx.enter_context(tc.tile_pool(name="pdp", bufs=2, space="PSUM"))

        rhs_tiles, lm_tiles, lf_tiles, hm_tiles, u_tiles = [], [], [], [], []

        def pre(c):
            r = rhsp.tile([K_RHS, TC * 128], F16, tag="rhs")
            nc.sync.dma_start(r[32:49, :],
                              rhsd_d[:, c * TC * 128:(c + 1) * TC * 128])
            rhs_tiles.append(r)
            m = lmp.tile([K_RHS, TC * H], F16, tag="lm")
            nc.sync.dma_start(m[:], lm_d[:, c * TC * H:(c + 1) * TC * H])
            lm_tiles.append(m)
            fi = lfp.tile([K_RHS, TC * H], F16, tag="lf")
            nc.sync.dma_start(fi[:], lf_d[:, c * TC * H:(c + 1) * TC * H])
            lf_tiles.append(fi)
            hh = hmp.tile([K_RHS, TC * 10], F16, tag="hm")
            nc.sync.dma_start(hh[:], hm_d[:, c * TC * 10:(c + 1) * TC * 10])
            hm_tiles.append(hh)
            u = usp.tile([10, TC * 128], F16, tag="u")
            u_tiles.append(u)

        def rslot(g):
            c, s = divmod(g, TC)
            return rhs_tiles[c][:, 128 * s:128 * (s + 1)]

        def hslot(g):
            c, s = divmod(g, TC)
            return rhs_tiles[c][0:32, 128 * s:128 * (s + 1)]

        def uslot(g):
            c, s = divmod(g, TC)
            return u_tiles[c][:, 128 * s:128 * (s + 1)]

        def lmsl(g):
            c, s = divmod(g, TC)
            return lm_tiles[c][:, H * s:H * (s + 1)]

        def lfsl(g):
            c, s = divmod(g, TC)
            return lf_tiles[c][:, H * s:H * (s + 1)]

        def hmsl(g):
            c, s = divmod(g, TC)
            return hm_tiles[c][:, 10 * s:10 * (s + 1)]

        pre(0)
        nc.tensor.matmul(pp[:], id_t[:], p0_t[:], start=True, stop=True,
                         skip_group_check=True)
        nc.tensor.matmul(ppH[:], idh_t[:], h0_t[:], start=True, stop=True,
                         skip_group_check=True)
        nc.vector.tensor_scalar_max(uslot(0), ppH[:], 0.0)

        for c in range(nchunks):
            if c + 1 < nchunks:
                pre(c + 1)
            u_t = u_tiles[c]
            po = pop.tile([128, 2 * TC], F32, tag="po", space="PSUM")

            for s in range(TC):
                g = c * TC + s
                if g < T - 1:
                    nc.scalar.activation(hslot(g), pp[:], AF.Tanh)

                if s % 4 == 0 and s > 0:
                    b = s // 4 - 1
                    for k in range(4):
                        sl = 4 * b + k
                        nc.tensor.matmul(
                            po[:, 2 * sl:2 * sl + 2],
                            u_t[:, 128 * sl:128 * (sl + 1)], w2_t[:],
                            start=True, stop=True)


                if g < T - 1:
                    if g >= 1:
                        nc.tensor.matmul(pp[:], lfsl(g - 1), rslot(g - 1),
                                         start=False, stop=False,
                                         skip_group_check=True)
                    nc.tensor.matmul(pp[:], lmsl(g), rslot(g),
                                     start=False, stop=True,
                                     skip_group_check=True)
                    nc.tensor.matmul(ppH[:], hmsl(g), rslot(g),
                                     start=False, stop=True,
                                     skip_group_check=True)
                    nc.vector.tensor_scalar_max(uslot(g + 1), ppH[:], 0.0)

            b = TC // 4 - 1
            for k in range(4):
                sl = 4 * b + k
                nc.tensor.matmul(po[:, 2 * sl:2 * sl + 2],
                                 u_t[:, 128 * sl:128 * (sl + 1)], w2_t[:],
                                 start=True, stop=True)
            osb_t = osbp.tile([128, 2 * TC], F32, tag="osb")
            nc.vector.tensor_copy(osb_t[:], po[:])
            nc.sync.dma_start(out_d[:, 2 * c * TC:2 * (c + 1) * TC],
                              osb_t[:])

    nc.compile()
    return nc


def _prep_inputs_v5(x, t, y0, Wr1, br1, Wr2, br2, W1, b1, W2, b2, T_=T):
    f16, f32, f64 = np.float16, np.float32, np.float64
    x = np.asarray(x, f32)
    tt = np.asarray(t, f32)
    y0 = np.asarray(y0, f32)
    Wr1 = np.asarray(Wr1, f32)
    Wy, We = Wr1[:S], Wr1[S:]
    Wr2 = np.asarray(Wr2, f32)
    br1 = np.asarray(br1, f32)
    br2 = np.asarray(br2, f32)
    W1 = np.asarray(W1, f32)
    b1 = np.asarray(b1, f32)
    W2 = np.asarray(W2, f32)
    dt = np.diff(tt).astype(f32)

    ex = np.zeros((T_, K_RHS, H), f64)
    Wt64 = f64(Wr2) @ f64(Wy)
    ex[:T_ - 1, 0:32] = dt[:, None, None].astype(f64) * Wt64[None]
    ex[:T_ - 1, 32:40] = f64(We)[None]
    ex[:T_ - 1, 40:48] = f64(We)[None]
    ex[:T_ - 1, 48] = dt[:, None].astype(f64) * (f64(br2) @ f64(Wy))[None]
    lm = ex.astype(f16)
    lf = (ex - lm.astype(f64)).astype(f16)

    M1 = f64(W1.T) @ np.linalg.pinv(f64(Wy.T))
    hm = np.zeros((T_, K_RHS, 10), f64)
    hm[:, 0:32] = ex[:, 0:32] @ M1.T
    hm[:, 48] = ex[:, 48] @ M1.T
    hm = hm.astype(f16)

    lm_s = np.ascontiguousarray(lm.transpose(1, 0, 2).reshape(K_RHS, T_ * H))
    lf_s = np.ascontiguousarray(lf.transpose(1, 0, 2).reshape(K_RHS, T_ * H))
    hm_s = np.ascontiguousarray(hm.transpose(1, 0, 2).reshape(K_RHS, T_ * 10))

    common = {
        "lm": lm_s, "lf": lf_s, "hm": hm_s,
        "w2f": W2.astype(f16),
        "id32": np.eye(H, dtype=f32),
        "id10": np.eye(10, dtype=f32),
    }
    in_maps = []
    for k in range(NCORES):
        sl = slice(k * BC, (k + 1) * BC)
        eT = np.ascontiguousarray(x[sl].transpose(2, 1, 0))
        d = eT[:, 1:, :] - eT[:, :-1, :]
        d_c = d.astype(f16)
        d_f = (d - d_c.astype(f32)).astype(f16)
        rhsd = np.ones((17, T_, BC), f16)
        rhsd[0:8, :T_ - 1] = d_c
        rhsd[0:8, T_ - 1] = 0
        rhsd[8:16, :T_ - 1] = d_f
        rhsd[8:16, T_ - 1] = 0
        p0 = (f64(Wy.T) @ f64(y0[sl].T) + f64(We.T) @ f64(eT[:, 0, :])
              + f64(br1)[:, None]).astype(f32)
        pre10 = (M1 @ f64(p0) - M1 @ f64(We.T) @ f64(eT[:, 0, :])
                 - (M1 @ f64(br1))[:, None] + f64(b1)[:, None]).astype(f32)
        in_maps.append({
            "rhsd": rhsd.reshape(17, T_ * BC),
            "p0t": p0,
            "pre10": pre10,
            **common,
        })
    return in_maps


# ---------------------------------------------------------------------------
# v6: v5 with the fine-correction matmul MERGED into the main matmul.
#
# The rhs stack widens to 98 rows: [h_t; h_{t-1}; d-parts; d-parts-dup].
# h_{t-1} is replicated into rows 32:64 of the NEXT slot by an off-chain
# DVE partition-offset copy (validated on HW); the d-parts are host-
# duplicated in the DMA stream. The combined lhsT carries main weights
# plus the fp16 rounding residuals (h-residual one step lagged, d/bias
# residuals current) -- one matmul per step updates the scan state with
# full fine correction. Serial chain: TANH -> single MM.
# ---------------------------------------------------------------------------

K2 = 98  # [h 32; h_lag 32; ones 1; d_c 8; d_f 8; d_c' 8; d_f' 8; ones' 1]
KH = 65  # head matmul contracts [h; h_lag(zero-weighted); ones]


def build_ode_nc_v6(T=T, TC=64):
    assert TC % 4 == 0 and T % TC == 0
    nchunks = T // TC
    F16 = mybir.dt.float16

    nc = bacc.Bacc()
    rhsd_d = nc.dram_tensor("rhsd", [34, T * 128], F16, kind="ExternalInput")
    lm_d = nc.dram_tensor("lm", [K2, T * H], F16, kind="ExternalInput")
    hm_d = nc.dram_tensor("hm", [KH, T * 10], F16, kind="ExternalInput")
    w2_d = nc.dram_tensor("w2f", [10, 2], F16, kind="ExternalInput")
    p0_d = nc.dram_tensor("p0t", [H, BC], F32, kind="ExternalInput")
    h0_d = nc.dram_tensor("pre10", [10, BC], F32, kind="ExternalInput")
    id_d = nc.dram_tensor("id32", [H, H], F32, kind="ExternalInput")
    idh_d = nc.dram_tensor("id10", [10, 10], F32, kind="ExternalInput")
    out_d = nc.dram_tensor("out", [BC, T * 2], F32, kind="ExternalOutput")

    with TileContext(nc) as tc, ExitStack() as ctx:
        cpool = ctx.enter_context(tc.tile_pool(name="consts", bufs=1))
        rhsp = ctx.enter_context(tc.tile_pool(name="rhs", bufs=3))
        lmp = ctx.enter_context(tc.tile_pool(name="lm", bufs=3))
        hmp = ctx.enter_context(tc.tile_pool(name="hm", bufs=3))
        usp = ctx.enter_context(tc.tile_pool(name="u", bufs=3))
        osbp = ctx.enter_context(tc.tile_pool(name="osb", bufs=2))
        ppp = ctx.enter_context(tc.tile_pool(name="ppp", bufs=1, space="PSUM"))
        pop = ctx.enter_context(tc.tile_pool(name="pop", bufs=2, space="PSUM"))

        def cload(name, shape, dram, dt_=F16):
            t_ = cpool.tile(shape, dt_, tag=name)
            nc.sync.dma_start(t_[:], dram[:])
            return t_

        w2_t = cload("w2", [10, 2], w2_d)
        p0_t = cload("p0", [H, BC], p0_d, F32)
        h0_t = cload("h0", [10, BC], h0_d, F32)
        id_t = cload("id32", [H, H], id_d, F32)
        idh_t = cload("id10", [10, 10], idh_d, F32)

        pp = ppp.tile([H, 128], F32, tag="pp", name="pp", space="PSUM")
        ppH = ppp.tile([10, 128], F32, tag="ppH", name="ppH", space="PSUM")
        pdp = ctx.enter_context(tc.tile_pool(name="pdp", bufs=2, space="PSUM"))

        rhs_tiles, lm_tiles, hm_tiles, u_tiles = [], [], [], []

        def pre(c):
            r = rhsp.tile([K2, TC * 128], F16, tag="rhs")
            nc.sync.dma_start(r[64:98, :],
                              rhsd_d[:, c * TC * 128:(c + 1) * TC * 128])
            rhs_tiles.append(r)
            m = lmp.tile([K2, TC * H], F16, tag="lm")
            nc.sync.dma_start(m[:], lm_d[:, c * TC * H:(c + 1) * TC * H])
            lm_tiles.append(m)
            hh = hmp.tile([KH, TC * 10], F16, tag="hm")
            nc.sync.dma_start(hh[:], hm_d[:, c * TC * 10:(c + 1) * TC * 10])
            hm_tiles.append(hh)
            u = usp.tile([10, TC * 128], F16, tag="u")
            u_tiles.append(u)

        def rslot(g):
            c, s = divmod(g, TC)
            return rhs_tiles[c][:, 128 * s:128 * (s + 1)]

        def hslot(g):
            c, s = divmod(g, TC)
            return rhs_tiles[c][0:32, 128 * s:128 * (s + 1)]

        def lagslot(g):
            c, s = divmod(g, TC)
            return rhs_tiles[c][32:64, 128 * s:128 * (s + 1)]

        def hdslot(g):
            c, s = divmod(g, TC)
            return rhs_tiles[c][0:KH, 128 * s:128 * (s + 1)]

        def uslot(g):
            c, s = divmod(g, TC)
            return u_tiles[c][:, 128 * s:128 * (s + 1)]

        def lmsl(g):
            c, s = divmod(g, TC)
            return lm_tiles[c][:, H * s:H * (s + 1)]

        def hmsl(g):
            c, s = divmod(g, TC)
            return hm_tiles[c][:, 10 * s:10 * (s + 1)]

        pre(0)
        nc.vector.memset(rhs_tiles[0][32:64, 0:128], 0.0)
        nc.tensor.matmul(pp[:], id_t[:], p0_t[:], start=True, stop=True,
                         skip_group_check=True)
        nc.tensor.matmul(ppH[:], idh_t[:], h0_t[:], start=True, stop=True,
                         skip_group_check=True)
        nc.vector.tensor_scalar_max(uslot(0), ppH[:], 0.0)

        for c in range(nchunks):
            if c + 1 < nchunks:
                pre(c + 1)
            u_t = u_tiles[c]
            po = pop.tile([128, 2 * TC], F32, tag="po", space="PSUM")

            for s in range(TC):
                g = c * TC + s
                if g < T - 1:
                    nc.scalar.activation(hslot(g), pp[:], AF.Tanh)
                    if g + 1 <= T - 2:
                        # replicate h_t into next slot's lag rows (off-chain)
                        nc.vector.tensor_copy(lagslot(g + 1), hslot(g))
                    # scan matmul first in the PE FIFO after TANH
                    nc.tensor.matmul(pp[:], lmsl(g), rslot(g),
                                     start=False, stop=True,
                                     skip_group_check=True)
                # HAM-warming filler tied to this step's h (runs in the
                # tanh-wait gap; keeps TensorE activity high)
                if g < T - 1:
                    pd = pdp.tile([128, 128], F32, tag="pd", space="PSUM")
                    nc.tensor.matmul(pd[:], hslot(g)[0:1, :],
                                     rslot(g)[0:1, :], start=True, stop=True)

                # head state update lags one step so it never delays MMm
                if 1 <= g:
                    nc.tensor.matmul(ppH[:], hmsl(g - 1), hdslot(g - 1),
                                     start=False, stop=True,
                                     skip_group_check=True)
                    nc.vector.tensor_scalar_max(uslot(g), ppH[:], 0.0)

                if s % 4 == 0 and s > 0:
                    b = s // 4 - 1
                    for k in range(4):
                        sl = 4 * b + k
                        nc.tensor.matmul(
                            po[:, 2 * sl:2 * sl + 2],
                            u_t[:, 128 * sl:128 * (sl + 1)], w2_t[:],
                            start=True, stop=True)


            b = TC // 4 - 1
            for k in range(4):
                sl = 4 * b + k
                nc.tensor.matmul(po[:, 2 * sl:2 * sl + 2],
                                 u_t[:, 128 * sl:128 * (sl + 1)], w2_t[:],
                                 start=True, stop=True)
            osb_t = osbp.tile([128, 2 * TC], F32, tag="osb")
            nc.vector.tensor_copy(osb_t[:], po[:])
            nc.sync.dma_start(out_d[:, 2 * c * TC:2 * (c + 1) * TC],
                              osb_t[:])

    nc.compile()
    return nc


def _prep_inputs_v6(x, t, y0, Wr1, br1, Wr2, br2, W1, b1, W2, b2, T_=T):
    f16, f32, f64 = np.float16, np.float32, np.float64
    x = np.asarray(x, f32)
    tt = np.asarray(t, f32)
    y0 = np.asarray(y0, f32)
    Wr1 = np.asarray(Wr1, f32)
    Wy, We = Wr1[:S], Wr1[S:]
    Wr2 = np.asarray(Wr2, f32)
    br1 = np.asarray(br1, f32)
    br2 = np.asarray(br2, f32)
    W1 = np.asarray(W1, f32)
    b1 = np.asarray(b1, f32)
    W2 = np.asarray(W2, f32)
    dt = np.diff(tt).astype(f32)

    # exact per-step weights in the 49-row basis, then split main/residual
    ex = np.zeros((T_, K_RHS, H), f64)
    Wt64 = f64(Wr2) @ f64(Wy)
    ex[:T_ - 1, 0:32] = dt[:, None, None].astype(f64) * Wt64[None]
    ex[:T_ - 1, 32:40] = f64(We)[None]
    ex[:T_ - 1, 40:48] = f64(We)[None]
    ex[:T_ - 1, 48] = dt[:, None].astype(f64) * (f64(br2) @ f64(Wy))[None]
    main = ex.astype(f16)
    resid = (ex - main.astype(f64)).astype(f16)

    # rows: 0:32 h | 32:64 h_lag | 64 ones | 65:73 d_c | 73:81 d_f
    #       | 81:89 d_c' | 89:97 d_f' | 97 ones'
    lm = np.zeros((T_, K2, H), f16)
    lm[:, 0:32] = main[:, 0:32]
    lm[1:, 32:64] = resid[:T_ - 1, 0:32]     # h-residual, lagged one step
    lm[:, 64] = main[:, 48]
    lm[:, 65:73] = main[:, 32:40]
    lm[:, 73:81] = main[:, 40:48]
    lm[:, 81:89] = resid[:, 32:40]
    lm[:, 89:97] = resid[:, 40:48]
    lm[:, 97] = resid[:, 48]

    M1 = f64(W1.T) @ np.linalg.pinv(f64(Wy.T))
    hm = np.zeros((T_, KH, 10), f64)
    hm[:, 0:32] = ex[:, 0:32] @ M1.T
    hm[:, 64] = ex[:, 48] @ M1.T
    hm = hm.astype(f16)

    lm_s = np.ascontiguousarray(lm.transpose(1, 0, 2).reshape(K2, T_ * H))
    hm_s = np.ascontiguousarray(hm.transpose(1, 0, 2).reshape(KH, T_ * 10))

    common = {
        "lm": lm_s, "hm": hm_s,
        "w2f": W2.astype(f16),
        "id32": np.eye(H, dtype=f32),
        "id10": np.eye(10, dtype=f32),
    }
    in_maps = []
    for k in range(NCORES):
        sl = slice(k * BC, (k + 1) * BC)
        eT = np.ascontiguousarray(x[sl].transpose(2, 1, 0))
        d = eT[:, 1:, :] - eT[:, :-1, :]
        d_c = d.astype(f16)
        d_f = (d - d_c.astype(f32)).astype(f16)
        rhsd = np.ones((34, T_, BC), f16)
        for base in (1, 17):                  # dest rows 65:81 and 81:97
            rhsd[base:base + 8, :T_ - 1] = d_c
            rhsd[base:base + 8, T_ - 1] = 0
            rhsd[base + 8:base + 16, :T_ - 1] = d_f
            rhsd[base + 8:base + 16, T_ - 1] = 0
        p0 = (f64(Wy.T) @ f64(y0[sl].T) + f64(We.T) @ f64(eT[:, 0, :])
              + f64(br1)[:, None]).astype(f32)
        pre10 = (M1 @ f64(p0) - M1 @ f64(We.T) @ f64(eT[:, 0, :])
                 - (M1 @ f64(br1))[:, None] + f64(b1)[:, None]).astype(f32)
        in_maps.append({
            "rhsd": rhsd.reshape(34, T_ * BC),
            "p0t": p0,
            "pre10": pre10,
            **common,
        })
    return in_maps


_NC_CACHE = {}


def kernel(x, t, y0, Wr1, br1, Wr2, br2, W1, b1, W2, b2):
    in_maps = _prep_inputs_v6(
        x, t, y0, Wr1, br1, Wr2, br2, W1, b1, W2, b2)
    key = ("v6e",)
    if key not in _NC_CACHE:
        _NC_CACHE[key] = build_ode_nc_v6(T=T, TC=64)
    nc = _NC_CACHE[key]
    res = bass_utils.run_bass_kernel_spmd(nc, in_maps,
                                          core_ids=list(range(NCORES)))
    outs = [res.results[k]["out"].reshape(BC, T, 2) for k in range(NCORES)]
    out = np.concatenate(outs, axis=0)
    b2 = np.asarray(b2, np.float32)
    if np.any(b2 != 0):
        out = out + b2[None, None, :]
    return out.astype(np.float32)

